# revision 27
# baseline (speedup 1.0000x reference)
"""Trainium2 Bass kernel for nn_Decoder (LSTM decoder + Luong attention + vocab proj).

Strategy (8 cores, data-parallel over batch, B_local = 4):
  phase 0: on-device prep per core:
    - embedding gather (indirect DMA) + Xw = X @ W1 + b precomputed for all steps,
      stored in DRAM as fp16 hi/lo pairs [512 tok, 2048].
    - keysT = (memory @ Wm')^T per batch -> fp16 hi/lo (Wm' = Wm/2, h2 convention)
    - fold attention out-proj into the recurrence:
        Wmod_h = Ul' + Wa_h' @ W2   (host pre-scales Ul, Wa_h by 1/2)
        Wmod_c = Wa_c @ W2          (g-gate cols pre-scaled x2)
      stored as fp16 hi/lo pairs.
    - Mproj[b] = mem[b] @ Wmod_c -> fp16 hi/lo.
    - step-1 correction corr = h_0 @ (Wa_h' @ W2) (since attn_0 = 0), fp32.
  phase 1: 128 sequential steps. All recurrence GEMMs run as 3-pass fp16
    hi/lo compensated matmuls (x_hi@w_hi + x_hi@w_lo + x_lo@w_hi): fp32-grade
    accuracy (measured 3.6e-7 per GEMM on HW) at bf16 PE rate (1 cyc/row vs
    4 for fp32). The cell state carries c2 = 2c and h2 = 2h so the sigmoid
    transform folds into fused scalar_tensor_tensor ops:
        c2' = 0.5*(tanh(zf/2)+1)*c2 + (tanh(zi/2)+1)*tanh(zg)
        h2  = (tanh(zo/2)+1)*tanh(c2'/2)
    z PSUM is split into 4 per-gate bank tiles so next step's Xw inject +
    score mask run on the PE while the current step's gates evaluate.
  phase 2: ctx materialized in batch from eT (fp16), attn = [H|CTX] @ Wa as
    fp16 GEMM, then logits = attn @ Wfc streaming Wfc fp32 via fast HW DMA and
    round-producing f32r tiles on the ACT engine; out rows are (t, b) tokens.
"""

import sys

for _p in ("/opt/trn_rl_repo",):
    if _p not in sys.path:
        sys.path.insert(0, _p)

import numpy as np

B, T, V, D, U = 32, 128, 32000, 256, 512
VO = V + 1
NCORES = 8
BL = B // NCORES  # 4
G = 4 * U  # 2048
NTOK = BL * T  # 512 tokens per core
HT_W = 4 * (T + 1)  # 516 columns per u-chunk in hT buffer

_cache = {}


def _build(n_steps=T):
    import concourse.bacc as bacc
    import concourse.bass as bass
    import concourse.mybir as mybir
    import concourse.tile as tile
    from concourse.masks import make_identity

    f32 = mybir.dt.float32
    f16 = mybir.dt.float16
    fr = mybir.dt.float32r  # full-rate PE path: phase-2 only (error hits logits directly)
    bf = mybir.dt.bfloat16
    i32 = mybir.dt.int32
    AX = mybir.AxisListType
    OP = mybir.AluOpType
    AF = mybir.ActivationFunctionType

    try:
        import concourse.tile_utils as _tu

        if getattr(_tu, "max_sbuf_usage", 0) < 204 * 1024:
            _tu.max_sbuf_usage = 204 * 1024
    except Exception:
        pass

    nc = bacc.Bacc(None, target_bir_lowering=False)

    tok_ids = nc.dram_tensor("tok_ids", [NTOK, 1], i32, kind="ExternalInput")
    mem_d = nc.dram_tensor("mem", [BL, T, U], f32, kind="ExternalInput")
    enc_ht_d = nc.dram_tensor("enc_ht", [U, BL], f32, kind="ExternalInput")  # 2*enc_h^T
    enc_c_d = nc.dram_tensor("enc_c", [BL, U], f32, kind="ExternalInput")   # 2*enc_c
    E_d = nc.dram_tensor("E", [V, D], f32, kind="ExternalInput")
    Wm_d = nc.dram_tensor("Wm", [U, U], f32, kind="ExternalInput")          # Wm/2
    W1_d = nc.dram_tensor("W1", [D, G], f32, kind="ExternalInput")
    W2_d = nc.dram_tensor("W2", [U, G], f32, kind="ExternalInput")
    Ul_d = nc.dram_tensor("Ul", [U, G], f32, kind="ExternalInput")          # Ul/2
    bl_d = nc.dram_tensor("bl", [1, G], f32, kind="ExternalInput")
    Wa_d = nc.dram_tensor("Wa", [2 * U, U], f32, kind="ExternalInput")      # [Wa_h/2; Wa_c]
    Wfc_d = nc.dram_tensor("Wfc", [U, VO], f32, kind="ExternalInput")
    bfc_d = nc.dram_tensor("bfc", [1, VO], f32, kind="ExternalInput")
    out_d = nc.dram_tensor("out", [NTOK, VO], f32, kind="ExternalOutput")

    n_chunks = (n_steps * BL + 127) // 128

    with tile.TileContext(nc) as tc:
        # ------------------------------------------------------------------
        # persistent pool
        # ------------------------------------------------------------------
        per_cm = tc.tile_pool(name="per", bufs=1)
        per = per_cm.__enter__()
        dram_cm = tc.tile_pool(name="dram", bufs=1, space="DRAM")
        dram = dram_cm.__enter__()

        wmh = [per.tile([128, G], f16, tag=f"wmh{k}", name=f"wmh{k}") for k in range(4)]
        wml = [per.tile([128, G], f16, tag=f"wml{k}", name=f"wml{k}") for k in range(4)]
        kTh = [per.tile([128, BL * T], f16, tag=f"kTh{j}", name=f"kTh{j}") for j in range(4)]
        kTl = [per.tile([128, BL * T], f16, tag=f"kTl{j}", name=f"kTl{j}") for j in range(4)]
        mpack = per.tile([128, BL * U], f16, tag="mpack")  # [t, (b,u)]; phase-2 only
        hth = per.tile([128, 4 * HT_W], f16, tag="hth")
        htl = per.tile([128, 4 * HT_W], f16, tag="htl")
        eTh = per.tile([128, 16 * T], f16, tag="eTh")
        eTl = per.tile([128, 16 * T], f16, tag="eTl")
        corr = per.tile([BL, G], f32, tag="corr")
        I4 = per.tile([4, 4], f32, tag="I4")        # f32: transpose identity
        I4b = per.tile([4, 4], bf, tag="I4b")       # bf16 lhsT for the mask matmul
        I4n = per.tile([4, 4], f32, tag="I4n")      # -I: corr inject (fp32)
        I128 = per.tile([128, 128], f32, tag="I128")
        I128h = per.tile([128, 128], f16, tag="I128h")
        ones1 = per.tile([1, 128], f32, tag="ones1")
        mneg = per.tile([BL, BL * T], bf, tag="mneg")
        mnegf = per.tile([BL, BL * T], f32, tag="mnegf")

        make_identity(nc, I4[:])
        make_identity(nc, I128[:])
        nc.vector.tensor_copy(I4b[:], I4[:])
        nc.vector.tensor_scalar_mul(I4n[:], I4[:], -1.0)
        nc.vector.tensor_copy(I128h[:], I128[:])
        onesf = per.tile([1, 128], f32, tag="onesf")
        nc.gpsimd.memset(onesf[:], 1.0)
        nc.vector.tensor_copy(ones1[:], onesf[:])
        # block-diagonal additive mask: 0 on own 128-block, -1e30 elsewhere.
        miot = per.tile([BL, BL * T], f32, tag="miot")
        nc.gpsimd.iota(
            miot[:], pattern=[[1, BL * T]], base=0, channel_multiplier=-T,
            allow_small_or_imprecise_dtypes=True,
        )
        ma = per.tile([BL, BL * T], f32, tag="ma")
        nc.vector.tensor_scalar(ma[:], miot[:], 0.0, None, op0=OP.is_ge)
        nc.vector.tensor_scalar(mnegf[:], miot[:], float(T - 1), None, op0=OP.is_le)
        nc.vector.tensor_tensor(ma[:], ma[:], mnegf[:], op=OP.mult)
        nc.vector.tensor_scalar(mneg[:], ma[:], -1.0, 1e30, op0=OP.add, op1=OP.mult)

        xw_hi_dram = dram.tile([NTOK, G], f16, name="xw_hi_dram")
        xw_lo_dram = dram.tile([NTOK, G], f16, name="xw_lo_dram")

        # ------------------------------------------------------------------
        # phase 0a: embedding gather + Xw = X @ W1 + bl (g cols x2) -> fp16
        # hi/lo in DRAM; memT (+ mpack fp16); keysT -> fp16 hi/lo
        # ------------------------------------------------------------------
        mproj_cm = tc.tile_pool(name="mprojp", bufs=1)
        mprojp = mproj_cm.__enter__()
        mph = [mprojp.tile([128, G], f16, tag=f"mph{b}", name=f"mph{b}") for b in range(BL)]
        mpl = [mprojp.tile([128, G], f16, tag=f"mpl{b}", name=f"mpl{b}") for b in range(BL)]
        mtv_cm = tc.tile_pool(name="mtvp", bufs=1)
        mtvp = mtv_cm.__enter__()
        mtv = [mtvp.tile([128, BL * 128], f32, tag=f"mtv{v}", name=f"mtv{v}") for v in range(4)]
        wmodc_cm = tc.tile_pool(name="wmodcp", bufs=1)
        wmodcp = wmodc_cm.__enter__()
        wmodc = [wmodcp.tile([128, G], f32, tag=f"wmodc{k}", name=f"wmodc{k}") for k in range(4)]

        with (
            tc.tile_pool(name="p0a", bufs=2) as p0a,
            tc.tile_pool(name="p0a1", bufs=1) as p0a1,
            tc.tile_pool(name="ps0", bufs=2, space="PSUM") as ps0,
        ):
            # init h2_0 = 2*enc_h (host-prescaled), fp16 hi/lo
            h0f = p0a1.tile([128, 4, BL], f32, tag="h0f")
            nc.sync.dma_start(h0f[:], enc_ht_d[:].rearrange("(j p) b -> p j b", j=4))
            h0hi = hth[:].rearrange("p (j s) -> p j s", j=4)[:, :, 0:BL]
            h0lo = htl[:].rearrange("p (j s) -> p j s", j=4)[:, :, 0:BL]
            nc.vector.tensor_copy(h0hi, h0f[:])
            nc.vector.tensor_tensor(h0lo, h0f[:], h0hi, op=OP.subtract)

            bls = p0a1.tile([1, G], f32, tag="bls")
            nc.sync.dma_start(bls[:], bl_d[:])
            # broadcast bl across partitions once (g cols x2 for the tanh trick)
            blsb = p0a1.tile([128, G], f32, tag="blsb")
            for q in range(4):
                pbl = ps0.tile([128, 512], f32, tag="pbl")
                nc.tensor.matmul(
                    pbl[:], ones1[:1, :], bls[:1, 512 * q : 512 * (q + 1)],
                    start=True, stop=True,
                )
                if q == 2:
                    nc.vector.tensor_scalar_mul(
                        blsb[:, 512 * q : 512 * (q + 1)], pbl[:], 2.0
                    )
                else:
                    nc.vector.tensor_copy(blsb[:, 512 * q : 512 * (q + 1)], pbl[:])
            xt = [p0a1.tile([128, NTOK], f32, tag=f"xt{k}", name=f"xt{k}") for k in range(2)]

            for c in range(NTOK // 128):
                ids_c = p0a.tile([128, 1], i32, tag="ids")
                nc.sync.dma_start(ids_c[:], tok_ids[128 * c : 128 * (c + 1)])
                x_c = p0a.tile([128, D], f32, tag="xc")
                nc.gpsimd.indirect_dma_start(
                    out=x_c[:],
                    out_offset=None,
                    in_=E_d[:],
                    in_offset=bass.IndirectOffsetOnAxis(ap=ids_c[:, :1], axis=0),
                )
                for k in range(2):
                    pt = ps0.tile([128, 128], f32, tag="pt0")
                    nc.tensor.transpose(pt[:], x_c[:, 128 * k : 128 * (k + 1)], I128[:])
                    nc.vector.tensor_copy(xt[k][:, 128 * c : 128 * (c + 1)], pt[:])

            for q in range(4):
                w1q = [
                    p0a.tile([128, 512], f32, tag="w1q", name=f"w1q{q}_{k}")
                    for k in range(2)
                ]
                for k in range(2):
                    nc.sync.dma_start(
                        w1q[k][:],
                        W1_d[128 * k : 128 * (k + 1), 512 * q : 512 * (q + 1)],
                    )
                for c in range(NTOK // 128):
                    pz0 = ps0.tile([128, 512], f32, tag="pz0")
                    for k in range(2):
                        nc.tensor.matmul(
                            pz0[:],
                            xt[k][:, 128 * c : 128 * (c + 1)],
                            w1q[k][:],
                            start=(k == 0),
                            stop=(k == 1),
                        )
                    st = p0a.tile([128, 512], f32, tag="xwst")
                    nc.vector.scalar_tensor_tensor(
                        st[:], pz0[:], 2.0 if q == 2 else 1.0,
                        blsb[:, 512 * q : 512 * (q + 1)],
                        op0=OP.mult, op1=OP.add,
                    )
                    sh = p0a.tile([128, 512], f16, tag="xwsh")
                    sl = p0a.tile([128, 512], f16, tag="xwsl")
                    nc.vector.tensor_copy(sh[:], st[:])
                    nc.vector.tensor_tensor(sl[:], st[:], sh[:], op=OP.subtract)
                    nc.sync.dma_start(
                        xw_hi_dram[128 * c : 128 * (c + 1), 512 * q : 512 * (q + 1)],
                        sh[:],
                    )
                    nc.sync.dma_start(
                        xw_lo_dram[128 * c : 128 * (c + 1), 512 * q : 512 * (q + 1)],
                        sl[:],
                    )

            # memT: mtv[vc][:, 128*b + t] = mem[b, t, 128*vc + v']; mpack fp16
            for b in range(BL):
                memf = p0a.tile([128, U], f32, tag="memf", name=f"memf{b}")
                nc.sync.dma_start(memf[:], mem_d[b])
                nc.vector.tensor_copy(mpack[:, U * b : U * (b + 1)], memf[:])
                for vc in range(4):
                    pt = ps0.tile([128, 128], f32, tag="pt0")
                    nc.tensor.transpose(
                        pt[:], memf[:, 128 * vc : 128 * (vc + 1)], I128[:]
                    )
                    nc.vector.tensor_copy(mtv[vc][:, 128 * b : 128 * (b + 1)], pt[:])

            # keysT (Wm pre-halved on host for the h2 convention) -> fp16 hi/lo
            wms = [p0a1.tile([128, U], f32, tag=f"wms{k}", name=f"wms{k}") for k in range(4)]
            for k in range(4):
                nc.sync.dma_start(wms[k][:], Wm_d[128 * k : 128 * (k + 1)])
            for j in range(4):
                for b in range(BL):
                    pk = ps0.tile([128, 128], f32, tag="pt0")
                    for vt in range(4):
                        nc.tensor.matmul(
                            pk[:],
                            wms[vt][:, 128 * j : 128 * (j + 1)],
                            mtv[vt][:, 128 * b : 128 * (b + 1)],
                            start=(vt == 0),
                            stop=(vt == 3),
                        )
                    hd = kTh[j][:, 128 * b : 128 * (b + 1)]
                    ld = kTl[j][:, 128 * b : 128 * (b + 1)]
                    nc.vector.tensor_copy(hd, pk[:])
                    nc.vector.tensor_tensor(ld, pk[:], hd, op=OP.subtract)

        # ------------------------------------------------------------------
        # phase 0c: Wmod_h = Ul' + Wa_h' @ W2 -> fp16 hi/lo (g cols x2);
        #           Wmod_c = Wa_c @ W2 (f32, feeds Mproj); corr
        # ------------------------------------------------------------------
        with (
            tc.tile_pool(name="p0c", bufs=1) as p0c,
            tc.tile_pool(name="p0cr", bufs=2) as p0cr,
            tc.tile_pool(name="p0w2", bufs=4) as p0w2,
            tc.tile_pool(name="ps0c", bufs=2, space="PSUM") as ps0c,
        ):
            was = [p0c.tile([128, U], f32, tag=f"was{k}", name=f"was{k}") for k in range(8)]
            for k in range(8):
                nc.sync.dma_start(was[k][:], Wa_d[128 * k : 128 * (k + 1)])
            wat = [p0c.tile([128, 2 * U], f32, tag=f"wat{q}", name=f"wat{q}") for q in range(4)]
            for k in range(8):
                for q in range(4):
                    pt = ps0c.tile([128, 128], f32, tag="ptc")
                    nc.tensor.transpose(
                        pt[:], was[k][:, 128 * q : 128 * (q + 1)], I128[:]
                    )
                    nc.vector.tensor_copy(wat[q][:, 128 * k : 128 * (k + 1)], pt[:])

            # enc_ht (=2*enc_h^T) as lhsT tiles: ehts[:, 4*kt + b]
            ehts = p0c.tile([128, 16], f32, tag="ehts")
            nc.sync.dma_start(
                ehts[:].rearrange("p (k b) -> p k b", k=4),
                enc_ht_d[:].rearrange("(k p) b -> p k b", k=4),
            )

            # corr: s = h2_0 @ Wa_h' = h_0 @ Wa_h ; corr = s @ W2 (g cols x2)
            ps_s = ps0c.tile([4, 512], f32, tag="ps_s")
            for kt in range(4):
                nc.tensor.matmul(
                    ps_s[:],
                    ehts[:, 4 * kt : 4 * kt + 4],
                    was[kt][:],
                    start=(kt == 0),
                    stop=(kt == 3),
                )
            s_sb = p0c.tile([4, 512], f32, tag="s_sb")
            nc.vector.tensor_copy(s_sb[:], ps_s[:])
            stT = p0c.tile([128, 16], f32, tag="stT")
            for j in range(4):
                pt = ps0c.tile([128, 16], f32, tag="pts")
                nc.tensor.transpose(
                    pt[:, 4 * j : 4 * j + 4], s_sb[:, 128 * j : 128 * (j + 1)], I4[:]
                )
                nc.vector.tensor_copy(stT[:, 4 * j : 4 * j + 4], pt[:, 4 * j : 4 * j + 4])

            # Mfold rows chunk mc (q-outer so W2 slices are loaded once)
            for q in range(4):
                w2q = [
                    p0w2.tile([128, 512], f32, tag="w2q", name=f"w2q{q}_{kt}")
                    for kt in range(4)
                ]
                for kt in range(4):
                    nc.sync.dma_start(
                        w2q[kt][:],
                        W2_d[128 * kt : 128 * (kt + 1), 512 * q : 512 * (q + 1)],
                    )
                for mc in range(8):
                    pm = ps0c.tile([128, 512], f32, tag="pm")
                    for kt in range(4):
                        nc.tensor.matmul(
                            pm[:],
                            wat[kt][:, 128 * mc : 128 * (mc + 1)],
                            w2q[kt][:],
                            start=(kt == 0),
                            stop=(kt == 3),
                        )
                    scl = 2.0 if q == 2 else 1.0
                    if mc < 4:
                        # h rows: Ul' chunk + Mfold (then g-scale) -> fp16 hi/lo
                        ul_t = p0cr.tile([128, 512], f32, tag="ul")
                        nc.sync.dma_start(
                            ul_t[:],
                            Ul_d[128 * mc : 128 * (mc + 1), 512 * q : 512 * (q + 1)],
                        )
                        sc = p0cr.tile([128, 512], f32, tag="sc")
                        if q == 2:
                            tmp = p0cr.tile([128, 512], f32, tag="gtmp")
                            nc.vector.tensor_tensor(tmp[:], pm[:], ul_t[:], op=OP.add)
                            nc.vector.tensor_scalar_mul(sc[:], tmp[:], 2.0)
                        else:
                            nc.vector.tensor_tensor(sc[:], pm[:], ul_t[:], op=OP.add)
                        hd = wmh[mc][:, 512 * q : 512 * (q + 1)]
                        ld = wml[mc][:, 512 * q : 512 * (q + 1)]
                        nc.vector.tensor_copy(hd, sc[:])
                        nc.vector.tensor_tensor(ld, sc[:], hd, op=OP.subtract)
                    else:
                        dst = wmodc[mc - 4][:, 512 * q : 512 * (q + 1)]
                        nc.scalar.activation(dst, pm[:], AF.Copy, bias=0.0, scale=scl)

                # corr chunk q while w2q is resident
                pc = ps0c.tile([4, 512], f32, tag="ps_s")
                for kt in range(4):
                    nc.tensor.matmul(
                        pc[:],
                        stT[:, 4 * kt : 4 * kt + 4],
                        w2q[kt][:],
                        start=(kt == 0),
                        stop=(kt == 3),
                    )
                nc.scalar.activation(
                    corr[:, 512 * q : 512 * (q + 1)],
                    pc[:],
                    AF.Copy,
                    bias=0.0,
                    scale=2.0 if q == 2 else 1.0,
                )

        # ------------------------------------------------------------------
        # phase 0d: Mproj[b] = mem[b] @ Wmod_c -> fp16 hi/lo
        # ------------------------------------------------------------------
        with tc.tile_pool(name="ps0d", bufs=2, space="PSUM") as ps0d:
            for b in range(BL):
                for q in range(4):
                    pm = ps0d.tile([128, 512], f32, tag="pmd")
                    for kt in range(4):
                        nc.tensor.matmul(
                            pm[:],
                            mtv[kt][:, 128 * b : 128 * (b + 1)],
                            wmodc[kt][:, 512 * q : 512 * (q + 1)],
                            start=(kt == 0),
                            stop=(kt == 3),
                        )
                    hd = mph[b][:, 512 * q : 512 * (q + 1)]
                    ld = mpl[b][:, 512 * q : 512 * (q + 1)]
                    nc.vector.tensor_copy(hd, pm[:])
                    nc.vector.tensor_tensor(ld, pm[:], hd, op=OP.subtract)
        wmodc_cm.__exit__(None, None, None)
        mtv_cm.__exit__(None, None, None)

        # ------------------------------------------------------------------
        # phase 1: the recurrence
        # ------------------------------------------------------------------
        with (
            tc.tile_pool(name="wk", bufs=1) as wk,
            tc.tile_pool(name="xwp", bufs=2) as xwp,
            tc.tile_pool(name="cst", bufs=2) as cst,
            tc.tile_pool(name="pz", bufs=5, space="PSUM") as pzp,
            tc.tile_pool(name="pat", bufs=2, space="PSUM") as patp,
            tc.tile_pool(name="ptr", bufs=1, space="PSUM") as ptrp,
        ):
            c2 = cst.tile([BL, U], f32, tag="c")
            nc.sync.dma_start(c2[:], enc_c_d[:])  # host passes 2*enc_c

            xwc = {}

            def load_xw_chunk(c):
                th_ = xwp.tile([128, G], f16, tag="xwh", name=f"xwh{c}")
                tl_ = xwp.tile([128, G], f16, tag="xwl", name=f"xwl{c}")
                rows = min(128, NTOK - 128 * c)
                nc.sync.dma_start(th_[:rows, :], xw_hi_dram[128 * c : 128 * c + rows])
                nc.sync.dma_start(tl_[:rows, :], xw_lo_dram[128 * c : 128 * c + rows])
                xwc[c] = (th_, tl_)

            load_xw_chunk(0)

            def hT_cols(tl, j, t0, ncols):
                v = tl[:].rearrange("p (j s) -> p j s", j=4)
                return v[:, j, 4 * t0 : 4 * t0 + ncols]

            def z_inject(t, pzq):
                """Xw hi/lo inject (+ t==1 corr): no dependency on h_{t-1};
                fills the PE while the previous step's gates evaluate."""
                ch = (t - 1) // 32
                row = 4 * ((t - 1) % 32)
                xh, xl = xwc[ch]
                for q in range(4):
                    zq = pzq[q][:]
                    nc.tensor.matmul(
                        zq, I128h[:, row : row + 4], xh[:, 512 * q : 512 * (q + 1)],
                        start=True, stop=False,
                    )
                    nc.tensor.matmul(
                        zq, I128h[:, row : row + 4], xl[:, 512 * q : 512 * (q + 1)],
                        start=False, stop=False,
                    )
                    if t == 1:
                        nc.tensor.matmul(
                            zq, I4n[:], corr[:, 512 * q : 512 * (q + 1)],
                            start=False, stop=False,
                        )

            def z_hpart(t, pzq, final):
                """h2_{t-1} @ Wmod_h, 3-pass fp16 hi/lo."""
                for kt in range(4):
                    hh = hT_cols(hth, kt, t - 1, 4)
                    hl = hT_cols(htl, kt, t - 1, 4)
                    for q in range(4):
                        zq = pzq[q][:]
                        nc.tensor.matmul(
                            zq, hh, wmh[kt][:, 512 * q : 512 * (q + 1)],
                            start=False, stop=False,
                        )
                        nc.tensor.matmul(
                            zq, hh, wml[kt][:, 512 * q : 512 * (q + 1)],
                            start=False, stop=False,
                        )
                        nc.tensor.matmul(
                            zq, hl, wmh[kt][:, 512 * q : 512 * (q + 1)],
                            start=False, stop=(final and kt == 3),
                        )

            def z_tail(t, pzq):
                """ctx contribution via alpha_{t-1} @ Mproj[b], 3-pass."""
                ec = 16 * (t - 2)
                for b in range(BL):
                    eh = eTh[:, ec + 4 * b : ec + 4 * b + 4]
                    el = eTl[:, ec + 4 * b : ec + 4 * b + 4]
                    for q in range(4):
                        zq = pzq[q][:]
                        nc.tensor.matmul(
                            zq, eh, mph[b][:, 512 * q : 512 * (q + 1)],
                            start=False, stop=False,
                        )
                        nc.tensor.matmul(
                            zq, eh, mpl[b][:, 512 * q : 512 * (q + 1)],
                            start=False, stop=False,
                        )
                        nc.tensor.matmul(
                            zq, el, mph[b][:, 512 * q : 512 * (q + 1)],
                            start=False, stop=(b == 3),
                        )

            def new_step_tiles(t):
                pzq = [
                    pzp.tile([BL, 512], f32, tag="pzq", name=f"pz{t}_{q}")
                    for q in range(4)
                ]
                psc = patp.tile([BL, BL * T], f32, tag="pat", name=f"psc{t}")
                return pzq, psc

            pzq_cur, psc_cur = new_step_tiles(1)
            z_inject(1, pzq_cur)
            nc.tensor.matmul(psc_cur[:], I4b[:], mneg[:], start=True, stop=False)
            z_hpart(1, pzq_cur, final=True)

            for t in range(1, n_steps + 1):
                if t % 32 == 2 and (t - 1) // 32 + 1 < n_chunks:
                    load_xw_chunk((t - 1) // 32 + 1)

                pzq, psc = pzq_cur, psc_cur

                # --- gates: per-q tanh chunks (i,f,g,o); f first ---
                th = wk.tile([BL, G], f32, tag="th")
                for q in (1, 0, 2, 3):
                    nc.scalar.activation(
                        th[:, 512 * q : 512 * (q + 1)], pzq[q][:],
                        AF.Tanh, bias=0.0, scale=0.5,
                    )

                # pre-issue t+1 PE work with no h_t dependency
                if t < n_steps:
                    pzq_cur, psc_cur = new_step_tiles(t + 1)
                    z_inject(t + 1, pzq_cur)
                    nc.tensor.matmul(
                        psc_cur[:], I4b[:], mneg[:], start=True, stop=False
                    )

                # --- cell update in the 2x basis ---
                # c2' = 0.5*(thf+1)*c2 + (thi+1)*tg ; h2 = (tho+1)*tanh(c2'/2)
                u4 = wk.tile([BL, U], f32, tag="u4")
                nc.vector.scalar_tensor_tensor(
                    u4[:], th[:, 512:1024], 1.0, c2[:], op0=OP.add, op1=OP.mult
                )
                v = wk.tile([BL, U], f32, tag="v")
                nc.vector.scalar_tensor_tensor(
                    v[:], th[:, 0:512], 1.0, th[:, 1024:1536],
                    op0=OP.add, op1=OP.mult,
                )
                c2n = cst.tile([BL, U], f32, tag="c")
                nc.vector.scalar_tensor_tensor(
                    c2n[:], u4[:], 0.5, v[:], op0=OP.mult, op1=OP.add
                )
                tc_ = wk.tile([BL, U], f32, tag="tc")
                nc.scalar.activation(tc_[:], c2n[:], AF.Tanh, bias=0.0, scale=0.5)
                h2 = wk.tile([BL, U], f32, tag="h")
                nc.vector.scalar_tensor_tensor(
                    h2[:], th[:, 1536:2048], 1.0, tc_[:], op0=OP.add, op1=OP.mult
                )
                c2 = c2n

                # --- hT hi/lo via PE transposes ---
                pht = ptrp.tile([128, 16], f32, tag="ptr")
                for j in range(4):
                    nc.tensor.transpose(
                        pht[:, 4 * j : 4 * j + 4], h2[:, 128 * j : 128 * (j + 1)], I4[:]
                    )
                phtv = pht[:].rearrange("p (j b) -> p j b", j=4)
                hiv = hth[:].rearrange("p (j s) -> p j s", j=4)[:, :, 4 * t : 4 * t + 4]
                lov = htl[:].rearrange("p (j s) -> p j s", j=4)[:, :, 4 * t : 4 * t + 4]
                nc.vector.tensor_copy(hiv, phtv)
                nc.vector.tensor_tensor(lov, phtv, hiv, op=OP.subtract)

                # --- score pairs [b, (b', t')] (mask pre-injected) ---
                for kt in range(4):
                    hh = hT_cols(hth, kt, t, 4)
                    hl = hT_cols(htl, kt, t, 4)
                    nc.tensor.matmul(psc[:], hh, kTh[kt][:], start=False, stop=False)
                    nc.tensor.matmul(psc[:], hh, kTl[kt][:], start=False, stop=False)
                    nc.tensor.matmul(
                        psc[:], hl, kTh[kt][:], start=False, stop=(kt == 3)
                    )

                # --- z_{t+1} h-part: fills the PE while softmax runs ---
                if t < n_steps:
                    z_hpart(t + 1, pzq_cur, final=False)

                # --- masked softmax straight off PSUM ---
                nmax = wk.tile([BL, 1], f32, tag="nmax")
                nc.vector.tensor_reduce(
                    nmax[:], psc[:], axis=AX.X, op=OP.max, negate=True
                )
                e = wk.tile([BL, BL * T], f32, tag="e")
                ssum = wk.tile([BL, 1], f32, tag="ssum")
                nc.scalar.activation(
                    e[:], psc[:], AF.Exp, bias=nmax[:, :1], scale=1.0,
                    accum_out=ssum[:, :1],
                )
                rec = wk.tile([BL, 1], f32, tag="rec")
                nc.vector.reciprocal(rec[:], ssum[:])
                e2 = wk.tile([BL, BL * T], f32, tag="e2")
                nc.vector.tensor_scalar(
                    e2[:], e[:], rec[:, :1], None, op0=OP.mult
                )

                # --- eT hi/lo blocks ---
                pet = ptrp.tile([128, 16], f32, tag="ptr")
                for q in range(BL):
                    nc.tensor.transpose(
                        pet[:, 4 * q : 4 * q + 4], e2[:, T * q : T * (q + 1)], I4[:]
                    )
                ehv = eTh[:, 16 * (t - 1) : 16 * t]
                elv = eTl[:, 16 * (t - 1) : 16 * t]
                nc.vector.tensor_copy(ehv, pet[:])
                nc.vector.tensor_tensor(elv, pet[:], ehv, op=OP.subtract)

                # --- z_{t+1} ctx part: needs eT of step t ---
                if t < n_steps:
                    z_tail(t + 1, pzq_cur)

        mproj_cm.__exit__(None, None, None)

        # ------------------------------------------------------------------
        # phase 2: ctxT from eT; attn = [H|CTX] @ Wa (fp16); logits = attn @ Wfc
        # ------------------------------------------------------------------
        with (
            tc.tile_pool(name="p2", bufs=1) as p2,
            tc.tile_pool(name="p2r", bufs=3) as p2r,
            tc.tile_pool(name="ps2", bufs=4, space="PSUM") as ps2,
        ):
            ntok = BL * n_steps

            # ctxT: ct2[j][:, 4*k + b] = ctx_{k+1}[b, 128j + u']
            ct2 = [p2.tile([128, NTOK], f16, tag=f"ct2{j}", name=f"ct2{j}") for j in range(4)]
            eview = eTh[:].rearrange("p (t s) -> p t s", s=16)
            for j in range(4):
                for b in range(BL):
                    pc2 = ps2.tile([128, T], f32, tag="pc2", bufs=2)
                    nc.tensor.matmul(
                        pc2[:, :n_steps],
                        mpack[:, U * b + 128 * j : U * b + 128 * (j + 1)],
                        eview[:, 0:n_steps, 4 * b + b],
                        start=True,
                        stop=True,
                    )
                    dst = ct2[j][:].rearrange("p (k b) -> p k b", b=4)[
                        :, 0:n_steps, b
                    ]
                    nc.vector.tensor_copy(dst, pc2[:, :n_steps])

            waxf = [p2.tile([128, U], f32, tag=f"waxf{k}", name=f"waxf{k}") for k in range(8)]
            wax = [p2.tile([128, U], f16, tag=f"wax{k}", name=f"wax{k}") for k in range(8)]
            for k in range(8):
                nc.sync.dma_start(waxf[k][:], Wa_d[128 * k : 128 * (k + 1)])
                nc.vector.tensor_copy(wax[k][:], waxf[k][:])
            att = [p2.tile([128, NTOK], fr, tag=f"att{j}", name=f"att{j}") for j in range(4)]
            for j in range(4):
                pa = ps2.tile([128, 512], f32, tag="pa", bufs=2)
                for kt in range(8):
                    if kt < 4:
                        src = hth[:].rearrange("p (jj s) -> p jj s", jj=4)[
                            :, kt, 4 : 4 + ntok
                        ]
                    else:
                        src = ct2[kt - 4][:, :ntok]
                    nc.tensor.matmul(
                        pa[:, :ntok],
                        wax[kt][:, 128 * j : 128 * (j + 1)],
                        src,
                        start=(kt == 0),
                        stop=(kt == 7),
                    )
                nc.vector.tensor_copy(att[j][:, :ntok], pa[:, :ntok])

            NCH = (VO + 511) // 512  # 63
            for nci in range(NCH):
                # last chunk overlaps the previous one so every chunk is a
                # full 512 wide (fp32r matmul needs aligned free dims)
                n0 = min(512 * nci, VO - 512)
                ncols = 512
                wff = p2r.tile([128, 4, 512], f32, tag="wff", bufs=4)
                nc.scalar.dma_start(
                    wff[:, :, :ncols],
                    Wfc_d[:, n0 : n0 + ncols].rearrange("(k p) n -> p k n", k=4),
                )
                # round-produce f32r on the (idle) ACT engine: fast HW DMA path
                # for the 65MB Wfc stream instead of the software cast-DMA
                wf = p2r.tile([128, 4, 512], fr, tag="wf")
                nc.scalar.activation(
                    wf[:].rearrange("p k n -> p (k n)"),
                    wff[:].rearrange("p k n -> p (k n)"),
                    AF.Copy, bias=0.0, scale=1.0,
                )
                bfc_t = p2r.tile([1, 512], f32, tag="bfc")
                nc.sync.dma_start(bfc_t[:1, :ncols], bfc_d[:1, n0 : n0 + ncols])
                # broadcast bias across partitions once per chunk (K=1 matmul)
                pbc = ps2.tile([128, 512], f32, tag="pbc", bufs=1)
                nc.tensor.matmul(
                    pbc[:, :ncols], ones1[:1, :], bfc_t[:1, :ncols],
                    start=True, stop=True,
                )
                bfcs = p2r.tile([128, 512], f32, tag="bfcs")
                nc.vector.tensor_copy(bfcs[:, :ncols], pbc[:, :ncols])
                for mt in range((ntok + 127) // 128):
                    mrows = min(128, ntok - 128 * mt)
                    pl = ps2.tile([128, 512], f32, tag="pl", bufs=3)
                    for kt in range(4):
                        nc.tensor.matmul(
                            pl[:mrows, :ncols],
                            att[kt][:, 128 * mt : 128 * mt + mrows],
                            wf[:, kt, :ncols],
                            start=(kt == 0),
                            stop=(kt == 3),
                        )
                    # bias folded into the PSUM->SBUF copy (per-column bcast add)
                    ot = p2r.tile([128, 512], f32, tag="ot")
                    nc.vector.scalar_tensor_tensor(
                        ot[:mrows, :ncols], pl[:mrows, :ncols], 1.0,
                        bfcs[:mrows, :ncols], op0=OP.mult, op1=OP.add,
                    )
                    nc.sync.dma_start(
                        out_d[128 * mt : 128 * mt + mrows, n0 : n0 + ncols],
                        ot[:mrows, :ncols],
                    )

        dram_cm.__exit__(None, None, None)
        per_cm.__exit__(None, None, None)

    nc.compile()
    return nc


def _shard_inputs(inputs, memory, enc_h, enc_c, E, Wm, W_lstm, U_lstm, b_lstm, Wa, Wfc, bfc):
    inputs = np.ascontiguousarray(inputs)
    # h2 = 2h convention: pre-halve everything h multiplies, double the carries
    Wa_mod = np.concatenate([0.5 * Wa[:U], Wa[U:]], axis=0)
    shared = {
        "E": np.ascontiguousarray(E, np.float32),
        "Wm": np.ascontiguousarray(0.5 * Wm, np.float32),
        "W1": np.ascontiguousarray(W_lstm[:D], np.float32),
        "W2": np.ascontiguousarray(W_lstm[D:], np.float32),
        "Ul": np.ascontiguousarray(0.5 * U_lstm, np.float32),
        "bl": np.ascontiguousarray(b_lstm.reshape(1, G), np.float32),
        "Wa": np.ascontiguousarray(Wa_mod, np.float32),
        "Wfc": np.ascontiguousarray(Wfc, np.float32),
        "bfc": np.ascontiguousarray(bfc.reshape(1, VO), np.float32),
    }
    in_maps = []
    for rk in range(NCORES):
        sl = slice(BL * rk, BL * (rk + 1))
        m = dict(shared)
        m["tok_ids"] = np.ascontiguousarray(
            inputs[sl].T.reshape(NTOK, 1), np.int32
        )
        m["mem"] = np.ascontiguousarray(memory[sl], np.float32)
        m["enc_ht"] = np.ascontiguousarray(2.0 * enc_h[sl].T, np.float32)
        m["enc_c"] = np.ascontiguousarray(2.0 * enc_c[sl], np.float32)
        in_maps.append(m)
    return in_maps


def kernel(**inputs):
    from concourse.bass_utils import run_bass_kernel_spmd

    if "nc" not in _cache:
        _cache["nc"] = _build(T)
    nc = _cache["nc"]

    in_maps = _shard_inputs(**inputs)
    res = run_bass_kernel_spmd(nc, in_maps, core_ids=list(range(NCORES)))
    outs = []
    for rk in range(NCORES):
        o = res.results[rk]["out"]  # [512, 32001], rows (t, b)
        outs.append(o.reshape(T, BL, VO).transpose(1, 0, 2))
    full = np.concatenate(outs, axis=0)  # [32, 128, 32001]
    return full.astype(np.float32)


# revision 28
# speedup vs baseline: 1.1624x; 1.1624x over previous
"""Trainium2 Bass kernel for nn_Decoder (LSTM decoder + Luong attention + vocab proj).

Strategy (8 cores, data-parallel over batch, B_local = 4):
  phase 0: on-device prep per core:
    - embedding gather (indirect DMA) + Xw = X @ W1 + b precomputed for all steps,
      stored in DRAM as fp16 hi/lo pairs [512 tok, 2048].
    - keysT = (memory @ Wm')^T per batch -> fp16 hi/lo (Wm' = Wm/2, h2 convention)
    - fold attention out-proj into the recurrence:
        Wmod_h = Ul' + Wa_h' @ W2   (host pre-scales Ul, Wa_h by 1/2)
        Wmod_c = Wa_c @ W2          (g-gate cols pre-scaled x2)
      stored as fp16 hi/lo pairs.
    - Mproj[b] = mem[b] @ Wmod_c -> fp16 hi/lo.
    - step-1 correction corr = h_0 @ (Wa_h' @ W2) (since attn_0 = 0), fp32.
  phase 1: 128 sequential steps. All recurrence GEMMs run as 3-pass fp16
    hi/lo compensated matmuls (x_hi@w_hi + x_hi@w_lo + x_lo@w_hi): fp32-grade
    accuracy (measured 3.6e-7 per GEMM on HW) at bf16 PE rate (1 cyc/row vs
    4 for fp32). The cell state carries c2 = 2c and h2 = 2h so the sigmoid
    transform folds into fused scalar_tensor_tensor ops:
        c2' = 0.5*(tanh(zf/2)+1)*c2 + (tanh(zi/2)+1)*tanh(zg)
        h2  = (tanh(zo/2)+1)*tanh(c2'/2)
    z PSUM is split into 4 per-gate bank tiles so next step's Xw inject +
    score mask run on the PE while the current step's gates evaluate.
  phase 2: ctx materialized in batch from eT (fp16), attn = [H|CTX] @ Wa as
    fp16 GEMM, then logits = attn @ Wfc streaming Wfc fp32 via fast HW DMA and
    round-producing f32r tiles on the ACT engine; out rows are (t, b) tokens.
"""

import sys

for _p in ("/opt/trn_rl_repo",):
    if _p not in sys.path:
        sys.path.insert(0, _p)

import numpy as np

B, T, V, D, U = 32, 128, 32000, 256, 512
VO = V + 1
NCORES = 8
BL = B // NCORES  # 4
G = 4 * U  # 2048
NTOK = BL * T  # 512 tokens per core
HT_W = 4 * (T + 1)  # 516 columns per u-chunk in hT buffer

_cache = {}


def _build(n_steps=T):
    import concourse.bacc as bacc
    import concourse.bass as bass
    import concourse.mybir as mybir
    import concourse.tile as tile
    from concourse.masks import make_identity

    f32 = mybir.dt.float32
    f16 = mybir.dt.float16
    fr = mybir.dt.float32r  # full-rate PE path: phase-2 only (error hits logits directly)
    bf = mybir.dt.bfloat16
    i32 = mybir.dt.int32
    AX = mybir.AxisListType
    OP = mybir.AluOpType
    AF = mybir.ActivationFunctionType

    try:
        import concourse.tile_utils as _tu

        if getattr(_tu, "max_sbuf_usage", 0) < 204 * 1024:
            _tu.max_sbuf_usage = 204 * 1024
    except Exception:
        pass

    nc = bacc.Bacc(None, target_bir_lowering=False)

    tok_ids = nc.dram_tensor("tok_ids", [NTOK, 1], i32, kind="ExternalInput")
    mem_d = nc.dram_tensor("mem", [BL, T, U], f32, kind="ExternalInput")
    enc_ht_d = nc.dram_tensor("enc_ht", [U, BL], f32, kind="ExternalInput")  # 2*enc_h^T
    enc_c_d = nc.dram_tensor("enc_c", [BL, U], f32, kind="ExternalInput")   # 2*enc_c
    E_d = nc.dram_tensor("E", [V, D], f32, kind="ExternalInput")
    Wm_d = nc.dram_tensor("Wm", [U, U], f32, kind="ExternalInput")          # Wm/2
    W1_d = nc.dram_tensor("W1", [D, G], f32, kind="ExternalInput")
    W2_d = nc.dram_tensor("W2", [U, G], f32, kind="ExternalInput")
    Ul_d = nc.dram_tensor("Ul", [U, G], f32, kind="ExternalInput")          # Ul/2
    bl_d = nc.dram_tensor("bl", [1, G], f32, kind="ExternalInput")
    Wa_d = nc.dram_tensor("Wa", [2 * U, U], f32, kind="ExternalInput")      # [Wa_h/2; Wa_c]
    Wfc_d = nc.dram_tensor("Wfc", [U, VO], f32, kind="ExternalInput")
    bfc_d = nc.dram_tensor("bfc", [1, VO], f32, kind="ExternalInput")
    out_d = nc.dram_tensor("out", [NTOK, VO], f32, kind="ExternalOutput")

    n_chunks = (n_steps * BL + 127) // 128

    with tile.TileContext(nc) as tc:
        # ------------------------------------------------------------------
        # persistent pool
        # ------------------------------------------------------------------
        per_cm = tc.tile_pool(name="per", bufs=1)
        per = per_cm.__enter__()
        dram_cm = tc.tile_pool(name="dram", bufs=1, space="DRAM")
        dram = dram_cm.__enter__()

        wmh = [per.tile([128, G], f16, tag=f"wmh{k}", name=f"wmh{k}") for k in range(4)]
        wml = [per.tile([128, G], f16, tag=f"wml{k}", name=f"wml{k}") for k in range(4)]
        kTh = [per.tile([128, BL * T], f16, tag=f"kTh{j}", name=f"kTh{j}") for j in range(4)]
        kTl = [per.tile([128, BL * T], f16, tag=f"kTl{j}", name=f"kTl{j}") for j in range(4)]
        mpack = per.tile([128, BL * U], f16, tag="mpack")  # [t, (b,u)]; phase-2 only
        hth = per.tile([128, 4 * HT_W], f16, tag="hth")
        htl = per.tile([128, 4 * HT_W], f16, tag="htl")
        eTh = per.tile([128, 16 * T], f16, tag="eTh")
        eTl = per.tile([128, 16 * T], f16, tag="eTl")
        corr = per.tile([BL, G], f32, tag="corr")
        I4 = per.tile([4, 4], f32, tag="I4")        # f32: transpose identity
        I4b = per.tile([4, 4], bf, tag="I4b")       # bf16 lhsT for the mask matmul
        I4n = per.tile([4, 4], f32, tag="I4n")      # -I: corr inject (fp32)
        I128 = per.tile([128, 128], f32, tag="I128")
        I128h = per.tile([128, 128], f16, tag="I128h")
        ones1 = per.tile([1, 128], f32, tag="ones1")
        mneg = per.tile([BL, BL * T], bf, tag="mneg")
        mnegf = per.tile([BL, BL * T], f32, tag="mnegf")

        make_identity(nc, I4[:])
        make_identity(nc, I128[:])
        nc.vector.tensor_copy(I4b[:], I4[:])
        nc.vector.tensor_scalar_mul(I4n[:], I4[:], -1.0)
        nc.vector.tensor_copy(I128h[:], I128[:])
        onesf = per.tile([1, 128], f32, tag="onesf")
        nc.gpsimd.memset(onesf[:], 1.0)
        nc.vector.tensor_copy(ones1[:], onesf[:])
        # block-diagonal additive mask: 0 on own 128-block, -1e30 elsewhere.
        miot = per.tile([BL, BL * T], f32, tag="miot")
        nc.gpsimd.iota(
            miot[:], pattern=[[1, BL * T]], base=0, channel_multiplier=-T,
            allow_small_or_imprecise_dtypes=True,
        )
        ma = per.tile([BL, BL * T], f32, tag="ma")
        nc.vector.tensor_scalar(ma[:], miot[:], 0.0, None, op0=OP.is_ge)
        nc.vector.tensor_scalar(mnegf[:], miot[:], float(T - 1), None, op0=OP.is_le)
        nc.vector.tensor_tensor(ma[:], ma[:], mnegf[:], op=OP.mult)
        nc.vector.tensor_scalar(mneg[:], ma[:], -1.0, 1e30, op0=OP.add, op1=OP.mult)

        xw_hi_dram = dram.tile([NTOK, G], f16, name="xw_hi_dram")
        xw_lo_dram = dram.tile([NTOK, G], f16, name="xw_lo_dram")

        # ------------------------------------------------------------------
        # phase 0a: embedding gather + Xw = X @ W1 + bl (g cols x2) -> fp16
        # hi/lo in DRAM; memT (+ mpack fp16); keysT -> fp16 hi/lo
        # ------------------------------------------------------------------
        mproj_cm = tc.tile_pool(name="mprojp", bufs=1)
        mprojp = mproj_cm.__enter__()
        mph = [mprojp.tile([128, G], f16, tag=f"mph{b}", name=f"mph{b}") for b in range(BL)]
        mpl = [mprojp.tile([128, G], f16, tag=f"mpl{b}", name=f"mpl{b}") for b in range(BL)]
        mtv_cm = tc.tile_pool(name="mtvp", bufs=1)
        mtvp = mtv_cm.__enter__()
        mtv = [mtvp.tile([128, BL * 128], f32, tag=f"mtv{v}", name=f"mtv{v}") for v in range(4)]
        wmodc_cm = tc.tile_pool(name="wmodcp", bufs=1)
        wmodcp = wmodc_cm.__enter__()
        wmodc = [wmodcp.tile([128, G], f32, tag=f"wmodc{k}", name=f"wmodc{k}") for k in range(4)]

        with (
            tc.tile_pool(name="p0a", bufs=2) as p0a,
            tc.tile_pool(name="p0a1", bufs=1) as p0a1,
            tc.tile_pool(name="ps0", bufs=2, space="PSUM") as ps0,
        ):
            # init h2_0 = 2*enc_h (host-prescaled), fp16 hi/lo
            h0f = p0a1.tile([128, 4, BL], f32, tag="h0f")
            nc.sync.dma_start(h0f[:], enc_ht_d[:].rearrange("(j p) b -> p j b", j=4))
            h0hi = hth[:].rearrange("p (j s) -> p j s", j=4)[:, :, 0:BL]
            h0lo = htl[:].rearrange("p (j s) -> p j s", j=4)[:, :, 0:BL]
            nc.vector.tensor_copy(h0hi, h0f[:])
            nc.vector.tensor_tensor(h0lo, h0f[:], h0hi, op=OP.subtract)

            bls = p0a1.tile([1, G], f32, tag="bls")
            nc.sync.dma_start(bls[:], bl_d[:])
            # broadcast bl across partitions once (g cols x2 for the tanh trick)
            blsb = p0a1.tile([128, G], f32, tag="blsb")
            for q in range(4):
                pbl = ps0.tile([128, 512], f32, tag="pbl")
                nc.tensor.matmul(
                    pbl[:], ones1[:1, :], bls[:1, 512 * q : 512 * (q + 1)],
                    start=True, stop=True,
                )
                if q == 2:
                    nc.vector.tensor_scalar_mul(
                        blsb[:, 512 * q : 512 * (q + 1)], pbl[:], 2.0
                    )
                else:
                    nc.vector.tensor_copy(blsb[:, 512 * q : 512 * (q + 1)], pbl[:])
            xt = [p0a1.tile([128, NTOK], f32, tag=f"xt{k}", name=f"xt{k}") for k in range(2)]

            for c in range(NTOK // 128):
                ids_c = p0a.tile([128, 1], i32, tag="ids")
                nc.sync.dma_start(ids_c[:], tok_ids[128 * c : 128 * (c + 1)])
                x_c = p0a.tile([128, D], f32, tag="xc")
                nc.gpsimd.indirect_dma_start(
                    out=x_c[:],
                    out_offset=None,
                    in_=E_d[:],
                    in_offset=bass.IndirectOffsetOnAxis(ap=ids_c[:, :1], axis=0),
                )
                for k in range(2):
                    pt = ps0.tile([128, 128], f32, tag="pt0")
                    nc.tensor.transpose(pt[:], x_c[:, 128 * k : 128 * (k + 1)], I128[:])
                    nc.vector.tensor_copy(xt[k][:, 128 * c : 128 * (c + 1)], pt[:])

            for q in range(4):
                w1q = [
                    p0a.tile([128, 512], f32, tag="w1q", name=f"w1q{q}_{k}")
                    for k in range(2)
                ]
                for k in range(2):
                    nc.sync.dma_start(
                        w1q[k][:],
                        W1_d[128 * k : 128 * (k + 1), 512 * q : 512 * (q + 1)],
                    )
                for c in range(NTOK // 128):
                    pz0 = ps0.tile([128, 512], f32, tag="pz0")
                    for k in range(2):
                        nc.tensor.matmul(
                            pz0[:],
                            xt[k][:, 128 * c : 128 * (c + 1)],
                            w1q[k][:],
                            start=(k == 0),
                            stop=(k == 1),
                        )
                    st = p0a.tile([128, 512], f32, tag="xwst")
                    nc.vector.scalar_tensor_tensor(
                        st[:], pz0[:], 2.0 if q == 2 else 1.0,
                        blsb[:, 512 * q : 512 * (q + 1)],
                        op0=OP.mult, op1=OP.add,
                    )
                    sh = p0a.tile([128, 512], f16, tag="xwsh")
                    sl = p0a.tile([128, 512], f16, tag="xwsl")
                    nc.vector.tensor_copy(sh[:], st[:])
                    nc.vector.tensor_tensor(sl[:], st[:], sh[:], op=OP.subtract)
                    nc.sync.dma_start(
                        xw_hi_dram[128 * c : 128 * (c + 1), 512 * q : 512 * (q + 1)],
                        sh[:],
                    )
                    nc.sync.dma_start(
                        xw_lo_dram[128 * c : 128 * (c + 1), 512 * q : 512 * (q + 1)],
                        sl[:],
                    )

            # memT: mtv[vc][:, 128*b + t] = mem[b, t, 128*vc + v']; mpack fp16
            for b in range(BL):
                memf = p0a.tile([128, U], f32, tag="memf", name=f"memf{b}")
                nc.sync.dma_start(memf[:], mem_d[b])
                nc.vector.tensor_copy(mpack[:, U * b : U * (b + 1)], memf[:])
                for vc in range(4):
                    pt = ps0.tile([128, 128], f32, tag="pt0")
                    nc.tensor.transpose(
                        pt[:], memf[:, 128 * vc : 128 * (vc + 1)], I128[:]
                    )
                    nc.vector.tensor_copy(mtv[vc][:, 128 * b : 128 * (b + 1)], pt[:])

            # keysT (Wm pre-halved on host for the h2 convention) -> fp16 hi/lo
            wms = [p0a1.tile([128, U], f32, tag=f"wms{k}", name=f"wms{k}") for k in range(4)]
            for k in range(4):
                nc.sync.dma_start(wms[k][:], Wm_d[128 * k : 128 * (k + 1)])
            for j in range(4):
                for b in range(BL):
                    pk = ps0.tile([128, 128], f32, tag="pt0")
                    for vt in range(4):
                        nc.tensor.matmul(
                            pk[:],
                            wms[vt][:, 128 * j : 128 * (j + 1)],
                            mtv[vt][:, 128 * b : 128 * (b + 1)],
                            start=(vt == 0),
                            stop=(vt == 3),
                        )
                    hd = kTh[j][:, 128 * b : 128 * (b + 1)]
                    ld = kTl[j][:, 128 * b : 128 * (b + 1)]
                    nc.vector.tensor_copy(hd, pk[:])
                    nc.vector.tensor_tensor(ld, pk[:], hd, op=OP.subtract)

        # ------------------------------------------------------------------
        # phase 0c: Wmod_h = Ul' + Wa_h' @ W2 -> fp16 hi/lo (g cols x2);
        #           Wmod_c = Wa_c @ W2 (f32, feeds Mproj); corr
        # ------------------------------------------------------------------
        with (
            tc.tile_pool(name="p0c", bufs=1) as p0c,
            tc.tile_pool(name="p0cr", bufs=2) as p0cr,
            tc.tile_pool(name="p0w2", bufs=4) as p0w2,
            tc.tile_pool(name="ps0c", bufs=2, space="PSUM") as ps0c,
        ):
            was = [p0c.tile([128, U], f32, tag=f"was{k}", name=f"was{k}") for k in range(8)]
            for k in range(8):
                nc.sync.dma_start(was[k][:], Wa_d[128 * k : 128 * (k + 1)])
            wat = [p0c.tile([128, 2 * U], f32, tag=f"wat{q}", name=f"wat{q}") for q in range(4)]
            for k in range(8):
                for q in range(4):
                    pt = ps0c.tile([128, 128], f32, tag="ptc")
                    nc.tensor.transpose(
                        pt[:], was[k][:, 128 * q : 128 * (q + 1)], I128[:]
                    )
                    nc.vector.tensor_copy(wat[q][:, 128 * k : 128 * (k + 1)], pt[:])

            # enc_ht (=2*enc_h^T) as lhsT tiles: ehts[:, 4*kt + b]
            ehts = p0c.tile([128, 16], f32, tag="ehts")
            nc.sync.dma_start(
                ehts[:].rearrange("p (k b) -> p k b", k=4),
                enc_ht_d[:].rearrange("(k p) b -> p k b", k=4),
            )

            # corr: s = h2_0 @ Wa_h' = h_0 @ Wa_h ; corr = s @ W2 (g cols x2)
            ps_s = ps0c.tile([4, 512], f32, tag="ps_s")
            for kt in range(4):
                nc.tensor.matmul(
                    ps_s[:],
                    ehts[:, 4 * kt : 4 * kt + 4],
                    was[kt][:],
                    start=(kt == 0),
                    stop=(kt == 3),
                )
            s_sb = p0c.tile([4, 512], f32, tag="s_sb")
            nc.vector.tensor_copy(s_sb[:], ps_s[:])
            stT = p0c.tile([128, 16], f32, tag="stT")
            for j in range(4):
                pt = ps0c.tile([128, 16], f32, tag="pts")
                nc.tensor.transpose(
                    pt[:, 4 * j : 4 * j + 4], s_sb[:, 128 * j : 128 * (j + 1)], I4[:]
                )
                nc.vector.tensor_copy(stT[:, 4 * j : 4 * j + 4], pt[:, 4 * j : 4 * j + 4])

            # Mfold rows chunk mc (q-outer so W2 slices are loaded once)
            for q in range(4):
                w2q = [
                    p0w2.tile([128, 512], f32, tag="w2q", name=f"w2q{q}_{kt}")
                    for kt in range(4)
                ]
                for kt in range(4):
                    nc.sync.dma_start(
                        w2q[kt][:],
                        W2_d[128 * kt : 128 * (kt + 1), 512 * q : 512 * (q + 1)],
                    )
                for mc in range(8):
                    pm = ps0c.tile([128, 512], f32, tag="pm")
                    for kt in range(4):
                        nc.tensor.matmul(
                            pm[:],
                            wat[kt][:, 128 * mc : 128 * (mc + 1)],
                            w2q[kt][:],
                            start=(kt == 0),
                            stop=(kt == 3),
                        )
                    scl = 2.0 if q == 2 else 1.0
                    if mc < 4:
                        # h rows: Ul' chunk + Mfold (then g-scale) -> fp16 hi/lo
                        ul_t = p0cr.tile([128, 512], f32, tag="ul")
                        nc.sync.dma_start(
                            ul_t[:],
                            Ul_d[128 * mc : 128 * (mc + 1), 512 * q : 512 * (q + 1)],
                        )
                        sc = p0cr.tile([128, 512], f32, tag="sc")
                        if q == 2:
                            tmp = p0cr.tile([128, 512], f32, tag="gtmp")
                            nc.vector.tensor_tensor(tmp[:], pm[:], ul_t[:], op=OP.add)
                            nc.vector.tensor_scalar_mul(sc[:], tmp[:], 2.0)
                        else:
                            nc.vector.tensor_tensor(sc[:], pm[:], ul_t[:], op=OP.add)
                        hd = wmh[mc][:, 512 * q : 512 * (q + 1)]
                        ld = wml[mc][:, 512 * q : 512 * (q + 1)]
                        nc.vector.tensor_copy(hd, sc[:])
                        nc.vector.tensor_tensor(ld, sc[:], hd, op=OP.subtract)
                    else:
                        dst = wmodc[mc - 4][:, 512 * q : 512 * (q + 1)]
                        nc.scalar.activation(dst, pm[:], AF.Copy, bias=0.0, scale=scl)

                # corr chunk q while w2q is resident
                pc = ps0c.tile([4, 512], f32, tag="ps_s")
                for kt in range(4):
                    nc.tensor.matmul(
                        pc[:],
                        stT[:, 4 * kt : 4 * kt + 4],
                        w2q[kt][:],
                        start=(kt == 0),
                        stop=(kt == 3),
                    )
                nc.scalar.activation(
                    corr[:, 512 * q : 512 * (q + 1)],
                    pc[:],
                    AF.Copy,
                    bias=0.0,
                    scale=2.0 if q == 2 else 1.0,
                )

        # ------------------------------------------------------------------
        # phase 0d: Mproj[b] = mem[b] @ Wmod_c -> fp16 hi/lo
        # ------------------------------------------------------------------
        with tc.tile_pool(name="ps0d", bufs=2, space="PSUM") as ps0d:
            for b in range(BL):
                for q in range(4):
                    pm = ps0d.tile([128, 512], f32, tag="pmd")
                    for kt in range(4):
                        nc.tensor.matmul(
                            pm[:],
                            mtv[kt][:, 128 * b : 128 * (b + 1)],
                            wmodc[kt][:, 512 * q : 512 * (q + 1)],
                            start=(kt == 0),
                            stop=(kt == 3),
                        )
                    hd = mph[b][:, 512 * q : 512 * (q + 1)]
                    ld = mpl[b][:, 512 * q : 512 * (q + 1)]
                    nc.vector.tensor_copy(hd, pm[:])
                    nc.vector.tensor_tensor(ld, pm[:], hd, op=OP.subtract)
        wmodc_cm.__exit__(None, None, None)
        mtv_cm.__exit__(None, None, None)

        # ------------------------------------------------------------------
        # phase 1: the recurrence
        # ------------------------------------------------------------------
        with (
            tc.tile_pool(name="wk", bufs=1) as wk,
            tc.tile_pool(name="xwp", bufs=2) as xwp,
            tc.tile_pool(name="cst", bufs=2) as cst,
            tc.tile_pool(name="pz", bufs=5, space="PSUM") as pzp,
            tc.tile_pool(name="pat", bufs=2, space="PSUM") as patp,
            tc.tile_pool(name="ptr", bufs=1, space="PSUM") as ptrp,
        ):
            c2 = cst.tile([BL, U], f32, tag="c")
            nc.sync.dma_start(c2[:], enc_c_d[:])  # host passes 2*enc_c

            xwc = {}

            def load_xw_chunk(c):
                th_ = xwp.tile([128, G], f16, tag="xwh", name=f"xwh{c}")
                tl_ = xwp.tile([128, G], f16, tag="xwl", name=f"xwl{c}")
                rows = min(128, NTOK - 128 * c)
                nc.sync.dma_start(th_[:rows, :], xw_hi_dram[128 * c : 128 * c + rows])
                nc.sync.dma_start(tl_[:rows, :], xw_lo_dram[128 * c : 128 * c + rows])
                xwc[c] = (th_, tl_)

            load_xw_chunk(0)

            def hT_cols(tl, j, t0, ncols):
                v = tl[:].rearrange("p (j s) -> p j s", j=4)
                return v[:, j, 4 * t0 : 4 * t0 + ncols]

            def z_inject(t, pzq):
                """Xw hi/lo inject (+ t==1 corr): no dependency on h_{t-1};
                fills the PE while the previous step's gates evaluate."""
                ch = (t - 1) // 32
                row = 4 * ((t - 1) % 32)
                xh, xl = xwc[ch]
                for q in range(4):
                    zq = pzq[q][:]
                    nc.tensor.matmul(
                        zq, I128h[:, row : row + 4], xh[:, 512 * q : 512 * (q + 1)],
                        start=True, stop=False,
                    )
                    nc.tensor.matmul(
                        zq, I128h[:, row : row + 4], xl[:, 512 * q : 512 * (q + 1)],
                        start=False, stop=False,
                    )
                    if t == 1:
                        nc.tensor.matmul(
                            zq, I4n[:], corr[:, 512 * q : 512 * (q + 1)],
                            start=False, stop=False,
                        )

            def z_hpart(t, pzq, final):
                """h2_{t-1} @ Wmod_h, 3-pass fp16 hi/lo."""
                for kt in range(4):
                    hh = hT_cols(hth, kt, t - 1, 4)
                    hl = hT_cols(htl, kt, t - 1, 4)
                    for q in range(4):
                        zq = pzq[q][:]
                        nc.tensor.matmul(
                            zq, hh, wmh[kt][:, 512 * q : 512 * (q + 1)],
                            start=False, stop=False,
                        )
                        nc.tensor.matmul(
                            zq, hh, wml[kt][:, 512 * q : 512 * (q + 1)],
                            start=False, stop=False,
                        )
                        nc.tensor.matmul(
                            zq, hl, wmh[kt][:, 512 * q : 512 * (q + 1)],
                            start=False, stop=(final and kt == 3),
                        )

            def z_tail(t, pzq):
                """ctx contribution via alpha_{t-1} @ Mproj[b], 3-pass."""
                ec = 16 * (t - 2)
                for b in range(BL):
                    eh = eTh[:, ec + 4 * b : ec + 4 * b + 4]
                    el = eTl[:, ec + 4 * b : ec + 4 * b + 4]
                    for q in range(4):
                        zq = pzq[q][:]
                        nc.tensor.matmul(
                            zq, eh, mph[b][:, 512 * q : 512 * (q + 1)],
                            start=False, stop=False,
                        )
                        nc.tensor.matmul(
                            zq, eh, mpl[b][:, 512 * q : 512 * (q + 1)],
                            start=False, stop=False,
                        )
                        nc.tensor.matmul(
                            zq, el, mph[b][:, 512 * q : 512 * (q + 1)],
                            start=False, stop=(b == 3),
                        )

            def new_step_tiles(t):
                pzq = [
                    pzp.tile([BL, 512], f32, tag="pzq", name=f"pz{t}_{q}")
                    for q in range(4)
                ]
                psc = patp.tile([BL, BL * T], f32, tag="pat", name=f"psc{t}")
                return pzq, psc

            pzq_cur, psc_cur = new_step_tiles(1)
            z_inject(1, pzq_cur)
            nc.tensor.matmul(psc_cur[:], I4b[:], mneg[:], start=True, stop=False)
            z_hpart(1, pzq_cur, final=True)

            for t in range(1, n_steps + 1):
                if t % 32 == 2 and (t - 1) // 32 + 1 < n_chunks:
                    load_xw_chunk((t - 1) // 32 + 1)

                pzq, psc = pzq_cur, psc_cur

                # --- gates: per-q tanh chunks (i,f,g,o); f first ---
                th = wk.tile([BL, G], f32, tag="th")
                for q in (1, 0, 2, 3):
                    nc.scalar.activation(
                        th[:, 512 * q : 512 * (q + 1)], pzq[q][:],
                        AF.Tanh, bias=0.0, scale=0.5,
                    )

                # pre-issue t+1 PE work with no h_t dependency
                if t < n_steps:
                    pzq_cur, psc_cur = new_step_tiles(t + 1)
                    z_inject(t + 1, pzq_cur)
                    nc.tensor.matmul(
                        psc_cur[:], I4b[:], mneg[:], start=True, stop=False
                    )

                # --- cell update in the 2x basis ---
                # c2' = 0.5*(thf+1)*c2 + (thi+1)*tg ; h2 = (tho+1)*tanh(c2'/2)
                u4 = wk.tile([BL, U], f32, tag="u4")
                nc.vector.scalar_tensor_tensor(
                    u4[:], th[:, 512:1024], 1.0, c2[:], op0=OP.add, op1=OP.mult
                )
                v = wk.tile([BL, U], f32, tag="v")
                nc.vector.scalar_tensor_tensor(
                    v[:], th[:, 0:512], 1.0, th[:, 1024:1536],
                    op0=OP.add, op1=OP.mult,
                )
                c2n = cst.tile([BL, U], f32, tag="c")
                nc.vector.scalar_tensor_tensor(
                    c2n[:], u4[:], 0.5, v[:], op0=OP.mult, op1=OP.add
                )
                tc_ = wk.tile([BL, U], f32, tag="tc")
                nc.scalar.activation(tc_[:], c2n[:], AF.Tanh, bias=0.0, scale=0.5)
                h2 = wk.tile([BL, U], f32, tag="h")
                nc.vector.scalar_tensor_tensor(
                    h2[:], th[:, 1536:2048], 1.0, tc_[:], op0=OP.add, op1=OP.mult
                )
                c2 = c2n

                # --- hT hi/lo via PE transposes ---
                pht = ptrp.tile([128, 16], f32, tag="ptr")
                for j in range(4):
                    nc.tensor.transpose(
                        pht[:, 4 * j : 4 * j + 4], h2[:, 128 * j : 128 * (j + 1)], I4[:]
                    )
                phtv = pht[:].rearrange("p (j b) -> p j b", j=4)
                hiv = hth[:].rearrange("p (j s) -> p j s", j=4)[:, :, 4 * t : 4 * t + 4]
                lov = htl[:].rearrange("p (j s) -> p j s", j=4)[:, :, 4 * t : 4 * t + 4]
                nc.vector.tensor_copy(hiv, phtv)
                nc.vector.tensor_tensor(lov, phtv, hiv, op=OP.subtract)

                # --- score pairs [b, (b', t')] (mask pre-injected) ---
                for kt in range(4):
                    hh = hT_cols(hth, kt, t, 4)
                    hl = hT_cols(htl, kt, t, 4)
                    nc.tensor.matmul(psc[:], hh, kTh[kt][:], start=False, stop=False)
                    nc.tensor.matmul(psc[:], hh, kTl[kt][:], start=False, stop=False)
                    nc.tensor.matmul(
                        psc[:], hl, kTh[kt][:], start=False, stop=(kt == 3)
                    )

                # --- z_{t+1} h-part: fills the PE while softmax runs ---
                if t < n_steps:
                    z_hpart(t + 1, pzq_cur, final=False)

                # --- masked softmax straight off PSUM ---
                nmax = wk.tile([BL, 1], f32, tag="nmax")
                nc.vector.tensor_reduce(
                    nmax[:], psc[:], axis=AX.X, op=OP.max, negate=True
                )
                e = wk.tile([BL, BL * T], f32, tag="e")
                ssum = wk.tile([BL, 1], f32, tag="ssum")
                nc.scalar.activation(
                    e[:], psc[:], AF.Exp, bias=nmax[:, :1], scale=1.0,
                    accum_out=ssum[:, :1],
                )
                rec = wk.tile([BL, 1], f32, tag="rec")
                nc.vector.reciprocal(rec[:], ssum[:])
                e2 = wk.tile([BL, BL * T], f32, tag="e2")
                nc.vector.tensor_scalar(
                    e2[:], e[:], rec[:, :1], None, op0=OP.mult
                )

                # --- eT hi/lo blocks ---
                pet = ptrp.tile([128, 16], f32, tag="ptr")
                for q in range(BL):
                    nc.tensor.transpose(
                        pet[:, 4 * q : 4 * q + 4], e2[:, T * q : T * (q + 1)], I4[:]
                    )
                ehv = eTh[:, 16 * (t - 1) : 16 * t]
                elv = eTl[:, 16 * (t - 1) : 16 * t]
                nc.vector.tensor_copy(ehv, pet[:])
                nc.vector.tensor_tensor(elv, pet[:], ehv, op=OP.subtract)

                # --- z_{t+1} ctx part: needs eT of step t ---
                if t < n_steps:
                    z_tail(t + 1, pzq_cur)

        mproj_cm.__exit__(None, None, None)

        # ------------------------------------------------------------------
        # phase 2: ctxT from eT; attn = [H|CTX] @ Wa (fp16); logits = attn @ Wfc
        # ------------------------------------------------------------------
        with (
            tc.tile_pool(name="p2", bufs=1) as p2,
            tc.tile_pool(name="p2r", bufs=3) as p2r,
            tc.tile_pool(name="ps2", bufs=4, space="PSUM") as ps2,
        ):
            ntok = BL * n_steps

            # ctxT: ct2[j][:, 4*k + b] = ctx_{k+1}[b, 128j + u']
            ct2 = [p2.tile([128, NTOK], f16, tag=f"ct2{j}", name=f"ct2{j}") for j in range(4)]
            eview = eTh[:].rearrange("p (t s) -> p t s", s=16)
            for j in range(4):
                for b in range(BL):
                    pc2 = ps2.tile([128, T], f32, tag="pc2", bufs=2)
                    nc.tensor.matmul(
                        pc2[:, :n_steps],
                        mpack[:, U * b + 128 * j : U * b + 128 * (j + 1)],
                        eview[:, 0:n_steps, 4 * b + b],
                        start=True,
                        stop=True,
                    )
                    dst = ct2[j][:].rearrange("p (k b) -> p k b", b=4)[
                        :, 0:n_steps, b
                    ]
                    nc.vector.tensor_copy(dst, pc2[:, :n_steps])

            waxf = [p2.tile([128, U], f32, tag=f"waxf{k}", name=f"waxf{k}") for k in range(8)]
            wax = [p2.tile([128, U], f16, tag=f"wax{k}", name=f"wax{k}") for k in range(8)]
            for k in range(8):
                nc.sync.dma_start(waxf[k][:], Wa_d[128 * k : 128 * (k + 1)])
                nc.vector.tensor_copy(wax[k][:], waxf[k][:])
            att = [p2.tile([128, NTOK], fr, tag=f"att{j}", name=f"att{j}") for j in range(4)]
            for j in range(4):
                pa = ps2.tile([128, 512], f32, tag="pa", bufs=2)
                for kt in range(8):
                    if kt < 4:
                        src = hth[:].rearrange("p (jj s) -> p jj s", jj=4)[
                            :, kt, 4 : 4 + ntok
                        ]
                    else:
                        src = ct2[kt - 4][:, :ntok]
                    nc.tensor.matmul(
                        pa[:, :ntok],
                        wax[kt][:, 128 * j : 128 * (j + 1)],
                        src,
                        start=(kt == 0),
                        stop=(kt == 7),
                    )
                nc.vector.tensor_copy(att[j][:, :ntok], pa[:, :ntok])

            NCH = (VO + 511) // 512  # 63
            for nci in range(NCH):
                # last chunk overlaps the previous one so every chunk is a
                # full 512 wide (fp32r matmul needs aligned free dims)
                n0 = min(512 * nci, VO - 512)
                ncols = 512
                wff = p2r.tile([128, 4, 512], f32, tag="wff", bufs=4)
                nc.sync.dma_start(
                    wff[:, :, :ncols],
                    Wfc_d[:, n0 : n0 + ncols].rearrange("(k p) n -> p k n", k=4),
                )
                # round-produce f32r on the (idle) ACT engine: fast HW DMA path
                # for the 65MB Wfc stream instead of the software cast-DMA
                wf = p2r.tile([128, 4, 512], fr, tag="wf")
                nc.scalar.activation(
                    wf[:].rearrange("p k n -> p (k n)"),
                    wff[:].rearrange("p k n -> p (k n)"),
                    AF.Copy, bias=0.0, scale=1.0,
                )
                bfc_t = p2r.tile([1, 512], f32, tag="bfc")
                nc.sync.dma_start(bfc_t[:1, :ncols], bfc_d[:1, n0 : n0 + ncols])
                # broadcast bias across partitions once per chunk (K=1 matmul)
                pbc = ps2.tile([128, 512], f32, tag="pbc", bufs=1)
                nc.tensor.matmul(
                    pbc[:, :ncols], ones1[:1, :], bfc_t[:1, :ncols],
                    start=True, stop=True,
                )
                bfcs = p2r.tile([128, 512], f32, tag="bfcs")
                nc.vector.tensor_copy(bfcs[:, :ncols], pbc[:, :ncols])
                for mt in range((ntok + 127) // 128):
                    mrows = min(128, ntok - 128 * mt)
                    pl = ps2.tile([128, 512], f32, tag="pl", bufs=3)
                    for kt in range(4):
                        nc.tensor.matmul(
                            pl[:mrows, :ncols],
                            att[kt][:, 128 * mt : 128 * mt + mrows],
                            wf[:, kt, :ncols],
                            start=(kt == 0),
                            stop=(kt == 3),
                        )
                    # bias folded into the PSUM->SBUF copy (per-column bcast add)
                    ot = p2r.tile([128, 512], f32, tag="ot")
                    nc.vector.scalar_tensor_tensor(
                        ot[:mrows, :ncols], pl[:mrows, :ncols], 1.0,
                        bfcs[:mrows, :ncols], op0=OP.mult, op1=OP.add,
                    )
                    nc.scalar.dma_start(
                        out_d[128 * mt : 128 * mt + mrows, n0 : n0 + ncols],
                        ot[:mrows, :ncols],
                    )

        dram_cm.__exit__(None, None, None)
        per_cm.__exit__(None, None, None)

    nc.compile()
    return nc


def _shard_inputs(inputs, memory, enc_h, enc_c, E, Wm, W_lstm, U_lstm, b_lstm, Wa, Wfc, bfc):
    inputs = np.ascontiguousarray(inputs)
    # h2 = 2h convention: pre-halve everything h multiplies, double the carries
    Wa_mod = np.concatenate([0.5 * Wa[:U], Wa[U:]], axis=0)
    shared = {
        "E": np.ascontiguousarray(E, np.float32),
        "Wm": np.ascontiguousarray(0.5 * Wm, np.float32),
        "W1": np.ascontiguousarray(W_lstm[:D], np.float32),
        "W2": np.ascontiguousarray(W_lstm[D:], np.float32),
        "Ul": np.ascontiguousarray(0.5 * U_lstm, np.float32),
        "bl": np.ascontiguousarray(b_lstm.reshape(1, G), np.float32),
        "Wa": np.ascontiguousarray(Wa_mod, np.float32),
        "Wfc": np.ascontiguousarray(Wfc, np.float32),
        "bfc": np.ascontiguousarray(bfc.reshape(1, VO), np.float32),
    }
    in_maps = []
    for rk in range(NCORES):
        sl = slice(BL * rk, BL * (rk + 1))
        m = dict(shared)
        m["tok_ids"] = np.ascontiguousarray(
            inputs[sl].T.reshape(NTOK, 1), np.int32
        )
        m["mem"] = np.ascontiguousarray(memory[sl], np.float32)
        m["enc_ht"] = np.ascontiguousarray(2.0 * enc_h[sl].T, np.float32)
        m["enc_c"] = np.ascontiguousarray(2.0 * enc_c[sl], np.float32)
        in_maps.append(m)
    return in_maps


def kernel(**inputs):
    from concourse.bass_utils import run_bass_kernel_spmd

    if "nc" not in _cache:
        _cache["nc"] = _build(T)
    nc = _cache["nc"]

    in_maps = _shard_inputs(**inputs)
    res = run_bass_kernel_spmd(nc, in_maps, core_ids=list(range(NCORES)))
    outs = []
    for rk in range(NCORES):
        o = res.results[rk]["out"]  # [512, 32001], rows (t, b)
        outs.append(o.reshape(T, BL, VO).transpose(1, 0, 2))
    full = np.concatenate(outs, axis=0)  # [32, 128, 32001]
    return full.astype(np.float32)


# revision 29
# speedup vs baseline: 1.1631x; 1.0006x over previous
"""Trainium2 Bass kernel for nn_Decoder (LSTM decoder + Luong attention + vocab proj).

Strategy (8 cores, data-parallel over batch, B_local = 4):
  phase 0: on-device prep per core:
    - embedding gather (indirect DMA) + Xw = X @ W1 + b precomputed for all steps,
      stored in DRAM as fp16 hi/lo pairs [512 tok, 2048].
    - keysT = (memory @ Wm')^T per batch -> fp16 hi/lo (Wm' = Wm/2, h2 convention)
    - fold attention out-proj into the recurrence:
        Wmod_h = Ul' + Wa_h' @ W2   (host pre-scales Ul, Wa_h by 1/2)
        Wmod_c = Wa_c @ W2          (g-gate cols pre-scaled x2)
      stored as fp16 hi/lo pairs.
    - Mproj[b] = mem[b] @ Wmod_c -> fp16 hi/lo.
    - step-1 correction corr = h_0 @ (Wa_h' @ W2) (since attn_0 = 0), fp32.
  phase 1: 128 sequential steps. All recurrence GEMMs run as 3-pass fp16
    hi/lo compensated matmuls (x_hi@w_hi + x_hi@w_lo + x_lo@w_hi): fp32-grade
    accuracy (measured 3.6e-7 per GEMM on HW) at bf16 PE rate (1 cyc/row vs
    4 for fp32). The cell state carries c2 = 2c and h2 = 2h so the sigmoid
    transform folds into fused scalar_tensor_tensor ops:
        c2' = 0.5*(tanh(zf/2)+1)*c2 + (tanh(zi/2)+1)*tanh(zg)
        h2  = (tanh(zo/2)+1)*tanh(c2'/2)
    z PSUM is split into 4 per-gate bank tiles so next step's Xw inject +
    score mask run on the PE while the current step's gates evaluate.
  phase 2: ctx materialized in batch from eT (fp16), attn = [H|CTX] @ Wa as
    fp16 GEMM, then logits = attn @ Wfc streaming Wfc fp32 via fast HW DMA and
    round-producing f32r tiles on the ACT engine; out rows are (t, b) tokens.
"""

import sys

for _p in ("/opt/trn_rl_repo",):
    if _p not in sys.path:
        sys.path.insert(0, _p)

import numpy as np

B, T, V, D, U = 32, 128, 32000, 256, 512
VO = V + 1
NCORES = 8
BL = B // NCORES  # 4
G = 4 * U  # 2048
NTOK = BL * T  # 512 tokens per core
HT_W = 4 * (T + 1)  # 516 columns per u-chunk in hT buffer

_cache = {}


def _build(n_steps=T):
    import concourse.bacc as bacc
    import concourse.bass as bass
    import concourse.mybir as mybir
    import concourse.tile as tile
    from concourse.masks import make_identity

    f32 = mybir.dt.float32
    f16 = mybir.dt.float16
    fr = mybir.dt.float32r  # full-rate PE path: phase-2 only (error hits logits directly)
    bf = mybir.dt.bfloat16
    i32 = mybir.dt.int32
    AX = mybir.AxisListType
    OP = mybir.AluOpType
    AF = mybir.ActivationFunctionType

    try:
        import concourse.tile_utils as _tu

        if getattr(_tu, "max_sbuf_usage", 0) < 204 * 1024:
            _tu.max_sbuf_usage = 204 * 1024
    except Exception:
        pass

    nc = bacc.Bacc(None, target_bir_lowering=False)

    tok_ids = nc.dram_tensor("tok_ids", [NTOK, 1], i32, kind="ExternalInput")
    mem_d = nc.dram_tensor("mem", [BL, T, U], f32, kind="ExternalInput")
    enc_ht_d = nc.dram_tensor("enc_ht", [U, BL], f32, kind="ExternalInput")  # 2*enc_h^T
    enc_c_d = nc.dram_tensor("enc_c", [BL, U], f32, kind="ExternalInput")   # 2*enc_c
    E_d = nc.dram_tensor("E", [V, D], f32, kind="ExternalInput")
    Wm_d = nc.dram_tensor("Wm", [U, U], f32, kind="ExternalInput")          # Wm/2
    W1_d = nc.dram_tensor("W1", [D, G], f32, kind="ExternalInput")
    W2_d = nc.dram_tensor("W2", [U, G], f32, kind="ExternalInput")
    Ul_d = nc.dram_tensor("Ul", [U, G], f32, kind="ExternalInput")          # Ul/2
    bl_d = nc.dram_tensor("bl", [1, G], f32, kind="ExternalInput")
    Wa_d = nc.dram_tensor("Wa", [2 * U, U], f32, kind="ExternalInput")      # [Wa_h/2; Wa_c]
    Wfc_d = nc.dram_tensor("Wfc", [U, VO], f32, kind="ExternalInput")
    bfc_d = nc.dram_tensor("bfc", [1, VO], f32, kind="ExternalInput")
    out_d = nc.dram_tensor("out", [NTOK, VO], f32, kind="ExternalOutput")

    n_chunks = (n_steps * BL + 127) // 128

    with tile.TileContext(nc) as tc:
        # ------------------------------------------------------------------
        # persistent pool
        # ------------------------------------------------------------------
        per_cm = tc.tile_pool(name="per", bufs=1)
        per = per_cm.__enter__()
        dram_cm = tc.tile_pool(name="dram", bufs=1, space="DRAM")
        dram = dram_cm.__enter__()

        wmh = [per.tile([128, G], f16, tag=f"wmh{k}", name=f"wmh{k}") for k in range(4)]
        wml = [per.tile([128, G], f16, tag=f"wml{k}", name=f"wml{k}") for k in range(4)]
        kTh = [per.tile([128, BL * T], f16, tag=f"kTh{j}", name=f"kTh{j}") for j in range(4)]
        kTl = [per.tile([128, BL * T], f16, tag=f"kTl{j}", name=f"kTl{j}") for j in range(4)]
        mpack = per.tile([128, BL * U], f16, tag="mpack")  # [t, (b,u)]; phase-2 only
        hth = per.tile([128, 4 * HT_W], f16, tag="hth")
        htl = per.tile([128, 4 * HT_W], f16, tag="htl")
        eTh = per.tile([128, 16 * T], f16, tag="eTh")
        eTl = per.tile([128, 16 * T], f16, tag="eTl")
        corr = per.tile([BL, G], f32, tag="corr")
        I4 = per.tile([4, 4], f32, tag="I4")        # f32: transpose identity
        I4b = per.tile([4, 4], bf, tag="I4b")       # bf16 lhsT for the mask matmul
        I4n = per.tile([4, 4], f32, tag="I4n")      # -I: corr inject (fp32)
        I128 = per.tile([128, 128], f32, tag="I128")
        I128h = per.tile([128, 128], f16, tag="I128h")
        ones1 = per.tile([1, 128], f32, tag="ones1")
        mneg = per.tile([BL, BL * T], bf, tag="mneg")
        mnegf = per.tile([BL, BL * T], f32, tag="mnegf")

        make_identity(nc, I4[:])
        make_identity(nc, I128[:])
        nc.vector.tensor_copy(I4b[:], I4[:])
        nc.vector.tensor_scalar_mul(I4n[:], I4[:], -1.0)
        nc.vector.tensor_copy(I128h[:], I128[:])
        onesf = per.tile([1, 128], f32, tag="onesf")
        nc.gpsimd.memset(onesf[:], 1.0)
        nc.vector.tensor_copy(ones1[:], onesf[:])
        # block-diagonal additive mask: 0 on own 128-block, -1e30 elsewhere.
        miot = per.tile([BL, BL * T], f32, tag="miot")
        nc.gpsimd.iota(
            miot[:], pattern=[[1, BL * T]], base=0, channel_multiplier=-T,
            allow_small_or_imprecise_dtypes=True,
        )
        ma = per.tile([BL, BL * T], f32, tag="ma")
        nc.vector.tensor_scalar(ma[:], miot[:], 0.0, None, op0=OP.is_ge)
        nc.vector.tensor_scalar(mnegf[:], miot[:], float(T - 1), None, op0=OP.is_le)
        nc.vector.tensor_tensor(ma[:], ma[:], mnegf[:], op=OP.mult)
        nc.vector.tensor_scalar(mneg[:], ma[:], -1.0, 1e30, op0=OP.add, op1=OP.mult)

        xw_hi_dram = dram.tile([NTOK, G], f16, name="xw_hi_dram")
        xw_lo_dram = dram.tile([NTOK, G], f16, name="xw_lo_dram")

        # ------------------------------------------------------------------
        # phase 0a: embedding gather + Xw = X @ W1 + bl (g cols x2) -> fp16
        # hi/lo in DRAM; memT (+ mpack fp16); keysT -> fp16 hi/lo
        # ------------------------------------------------------------------
        mproj_cm = tc.tile_pool(name="mprojp", bufs=1)
        mprojp = mproj_cm.__enter__()
        mph = [mprojp.tile([128, G], f16, tag=f"mph{b}", name=f"mph{b}") for b in range(BL)]
        mpl = [mprojp.tile([128, G], f16, tag=f"mpl{b}", name=f"mpl{b}") for b in range(BL)]
        mtv_cm = tc.tile_pool(name="mtvp", bufs=1)
        mtvp = mtv_cm.__enter__()
        mtv = [mtvp.tile([128, BL * 128], f32, tag=f"mtv{v}", name=f"mtv{v}") for v in range(4)]
        wmodc_cm = tc.tile_pool(name="wmodcp", bufs=1)
        wmodcp = wmodc_cm.__enter__()
        wmodc = [wmodcp.tile([128, G], f32, tag=f"wmodc{k}", name=f"wmodc{k}") for k in range(4)]

        with (
            tc.tile_pool(name="p0a", bufs=2) as p0a,
            tc.tile_pool(name="p0a1", bufs=1) as p0a1,
            tc.tile_pool(name="ps0", bufs=2, space="PSUM") as ps0,
        ):
            # init h2_0 = 2*enc_h (host-prescaled), fp16 hi/lo
            h0f = p0a1.tile([128, 4, BL], f32, tag="h0f")
            nc.sync.dma_start(h0f[:], enc_ht_d[:].rearrange("(j p) b -> p j b", j=4))
            h0hi = hth[:].rearrange("p (j s) -> p j s", j=4)[:, :, 0:BL]
            h0lo = htl[:].rearrange("p (j s) -> p j s", j=4)[:, :, 0:BL]
            nc.vector.tensor_copy(h0hi, h0f[:])
            nc.vector.tensor_tensor(h0lo, h0f[:], h0hi, op=OP.subtract)

            bls = p0a1.tile([1, G], f32, tag="bls")
            nc.sync.dma_start(bls[:], bl_d[:])
            # broadcast bl across partitions once (g cols x2 for the tanh trick)
            blsb = p0a1.tile([128, G], f32, tag="blsb")
            for q in range(4):
                pbl = ps0.tile([128, 512], f32, tag="pbl")
                nc.tensor.matmul(
                    pbl[:], ones1[:1, :], bls[:1, 512 * q : 512 * (q + 1)],
                    start=True, stop=True,
                )
                if q == 2:
                    nc.vector.tensor_scalar_mul(
                        blsb[:, 512 * q : 512 * (q + 1)], pbl[:], 2.0
                    )
                else:
                    nc.vector.tensor_copy(blsb[:, 512 * q : 512 * (q + 1)], pbl[:])
            xt = [p0a1.tile([128, NTOK], f32, tag=f"xt{k}", name=f"xt{k}") for k in range(2)]

            for c in range(NTOK // 128):
                ids_c = p0a.tile([128, 1], i32, tag="ids")
                nc.sync.dma_start(ids_c[:], tok_ids[128 * c : 128 * (c + 1)])
                x_c = p0a.tile([128, D], f32, tag="xc")
                nc.gpsimd.indirect_dma_start(
                    out=x_c[:],
                    out_offset=None,
                    in_=E_d[:],
                    in_offset=bass.IndirectOffsetOnAxis(ap=ids_c[:, :1], axis=0),
                )
                for k in range(2):
                    pt = ps0.tile([128, 128], f32, tag="pt0")
                    nc.tensor.transpose(pt[:], x_c[:, 128 * k : 128 * (k + 1)], I128[:])
                    nc.vector.tensor_copy(xt[k][:, 128 * c : 128 * (c + 1)], pt[:])

            for q in range(4):
                w1q = [
                    p0a.tile([128, 512], f32, tag="w1q", name=f"w1q{q}_{k}")
                    for k in range(2)
                ]
                for k in range(2):
                    nc.sync.dma_start(
                        w1q[k][:],
                        W1_d[128 * k : 128 * (k + 1), 512 * q : 512 * (q + 1)],
                    )
                for c in range(NTOK // 128):
                    pz0 = ps0.tile([128, 512], f32, tag="pz0")
                    for k in range(2):
                        nc.tensor.matmul(
                            pz0[:],
                            xt[k][:, 128 * c : 128 * (c + 1)],
                            w1q[k][:],
                            start=(k == 0),
                            stop=(k == 1),
                        )
                    st = p0a.tile([128, 512], f32, tag="xwst")
                    nc.vector.scalar_tensor_tensor(
                        st[:], pz0[:], 2.0 if q == 2 else 1.0,
                        blsb[:, 512 * q : 512 * (q + 1)],
                        op0=OP.mult, op1=OP.add,
                    )
                    sh = p0a.tile([128, 512], f16, tag="xwsh")
                    sl = p0a.tile([128, 512], f16, tag="xwsl")
                    nc.vector.tensor_copy(sh[:], st[:])
                    nc.vector.tensor_tensor(sl[:], st[:], sh[:], op=OP.subtract)
                    nc.sync.dma_start(
                        xw_hi_dram[128 * c : 128 * (c + 1), 512 * q : 512 * (q + 1)],
                        sh[:],
                    )
                    nc.sync.dma_start(
                        xw_lo_dram[128 * c : 128 * (c + 1), 512 * q : 512 * (q + 1)],
                        sl[:],
                    )

            # memT: mtv[vc][:, 128*b + t] = mem[b, t, 128*vc + v']; mpack fp16
            for b in range(BL):
                memf = p0a.tile([128, U], f32, tag="memf", name=f"memf{b}")
                nc.sync.dma_start(memf[:], mem_d[b])
                nc.vector.tensor_copy(mpack[:, U * b : U * (b + 1)], memf[:])
                for vc in range(4):
                    pt = ps0.tile([128, 128], f32, tag="pt0")
                    nc.tensor.transpose(
                        pt[:], memf[:, 128 * vc : 128 * (vc + 1)], I128[:]
                    )
                    nc.vector.tensor_copy(mtv[vc][:, 128 * b : 128 * (b + 1)], pt[:])

            # keysT (Wm pre-halved on host for the h2 convention) -> fp16 hi/lo
            wms = [p0a1.tile([128, U], f32, tag=f"wms{k}", name=f"wms{k}") for k in range(4)]
            for k in range(4):
                nc.sync.dma_start(wms[k][:], Wm_d[128 * k : 128 * (k + 1)])
            for j in range(4):
                for b in range(BL):
                    pk = ps0.tile([128, 128], f32, tag="pt0")
                    for vt in range(4):
                        nc.tensor.matmul(
                            pk[:],
                            wms[vt][:, 128 * j : 128 * (j + 1)],
                            mtv[vt][:, 128 * b : 128 * (b + 1)],
                            start=(vt == 0),
                            stop=(vt == 3),
                        )
                    hd = kTh[j][:, 128 * b : 128 * (b + 1)]
                    ld = kTl[j][:, 128 * b : 128 * (b + 1)]
                    nc.vector.tensor_copy(hd, pk[:])
                    nc.vector.tensor_tensor(ld, pk[:], hd, op=OP.subtract)

        # ------------------------------------------------------------------
        # phase 0c: Wmod_h = Ul' + Wa_h' @ W2 -> fp16 hi/lo (g cols x2);
        #           Wmod_c = Wa_c @ W2 (f32, feeds Mproj); corr
        # ------------------------------------------------------------------
        with (
            tc.tile_pool(name="p0c", bufs=1) as p0c,
            tc.tile_pool(name="p0cr", bufs=2) as p0cr,
            tc.tile_pool(name="p0w2", bufs=4) as p0w2,
            tc.tile_pool(name="ps0c", bufs=2, space="PSUM") as ps0c,
        ):
            was = [p0c.tile([128, U], f32, tag=f"was{k}", name=f"was{k}") for k in range(8)]
            for k in range(8):
                nc.sync.dma_start(was[k][:], Wa_d[128 * k : 128 * (k + 1)])
            wat = [p0c.tile([128, 2 * U], f32, tag=f"wat{q}", name=f"wat{q}") for q in range(4)]
            for k in range(8):
                for q in range(4):
                    pt = ps0c.tile([128, 128], f32, tag="ptc")
                    nc.tensor.transpose(
                        pt[:], was[k][:, 128 * q : 128 * (q + 1)], I128[:]
                    )
                    nc.vector.tensor_copy(wat[q][:, 128 * k : 128 * (k + 1)], pt[:])

            # enc_ht (=2*enc_h^T) as lhsT tiles: ehts[:, 4*kt + b]
            ehts = p0c.tile([128, 16], f32, tag="ehts")
            nc.sync.dma_start(
                ehts[:].rearrange("p (k b) -> p k b", k=4),
                enc_ht_d[:].rearrange("(k p) b -> p k b", k=4),
            )

            # corr: s = h2_0 @ Wa_h' = h_0 @ Wa_h ; corr = s @ W2 (g cols x2)
            ps_s = ps0c.tile([4, 512], f32, tag="ps_s")
            for kt in range(4):
                nc.tensor.matmul(
                    ps_s[:],
                    ehts[:, 4 * kt : 4 * kt + 4],
                    was[kt][:],
                    start=(kt == 0),
                    stop=(kt == 3),
                )
            s_sb = p0c.tile([4, 512], f32, tag="s_sb")
            nc.vector.tensor_copy(s_sb[:], ps_s[:])
            stT = p0c.tile([128, 16], f32, tag="stT")
            for j in range(4):
                pt = ps0c.tile([128, 16], f32, tag="pts")
                nc.tensor.transpose(
                    pt[:, 4 * j : 4 * j + 4], s_sb[:, 128 * j : 128 * (j + 1)], I4[:]
                )
                nc.vector.tensor_copy(stT[:, 4 * j : 4 * j + 4], pt[:, 4 * j : 4 * j + 4])

            # Mfold rows chunk mc (q-outer so W2 slices are loaded once)
            for q in range(4):
                w2q = [
                    p0w2.tile([128, 512], f32, tag="w2q", name=f"w2q{q}_{kt}")
                    for kt in range(4)
                ]
                for kt in range(4):
                    nc.sync.dma_start(
                        w2q[kt][:],
                        W2_d[128 * kt : 128 * (kt + 1), 512 * q : 512 * (q + 1)],
                    )
                for mc in range(8):
                    pm = ps0c.tile([128, 512], f32, tag="pm")
                    for kt in range(4):
                        nc.tensor.matmul(
                            pm[:],
                            wat[kt][:, 128 * mc : 128 * (mc + 1)],
                            w2q[kt][:],
                            start=(kt == 0),
                            stop=(kt == 3),
                        )
                    scl = 2.0 if q == 2 else 1.0
                    if mc < 4:
                        # h rows: Ul' chunk + Mfold (then g-scale) -> fp16 hi/lo
                        ul_t = p0cr.tile([128, 512], f32, tag="ul")
                        nc.sync.dma_start(
                            ul_t[:],
                            Ul_d[128 * mc : 128 * (mc + 1), 512 * q : 512 * (q + 1)],
                        )
                        sc = p0cr.tile([128, 512], f32, tag="sc")
                        if q == 2:
                            tmp = p0cr.tile([128, 512], f32, tag="gtmp")
                            nc.vector.tensor_tensor(tmp[:], pm[:], ul_t[:], op=OP.add)
                            nc.vector.tensor_scalar_mul(sc[:], tmp[:], 2.0)
                        else:
                            nc.vector.tensor_tensor(sc[:], pm[:], ul_t[:], op=OP.add)
                        hd = wmh[mc][:, 512 * q : 512 * (q + 1)]
                        ld = wml[mc][:, 512 * q : 512 * (q + 1)]
                        nc.vector.tensor_copy(hd, sc[:])
                        nc.vector.tensor_tensor(ld, sc[:], hd, op=OP.subtract)
                    else:
                        dst = wmodc[mc - 4][:, 512 * q : 512 * (q + 1)]
                        nc.scalar.activation(dst, pm[:], AF.Copy, bias=0.0, scale=scl)

                # corr chunk q while w2q is resident
                pc = ps0c.tile([4, 512], f32, tag="ps_s")
                for kt in range(4):
                    nc.tensor.matmul(
                        pc[:],
                        stT[:, 4 * kt : 4 * kt + 4],
                        w2q[kt][:],
                        start=(kt == 0),
                        stop=(kt == 3),
                    )
                nc.scalar.activation(
                    corr[:, 512 * q : 512 * (q + 1)],
                    pc[:],
                    AF.Copy,
                    bias=0.0,
                    scale=2.0 if q == 2 else 1.0,
                )

        # ------------------------------------------------------------------
        # phase 0d: Mproj[b] = mem[b] @ Wmod_c -> fp16 hi/lo
        # ------------------------------------------------------------------
        with tc.tile_pool(name="ps0d", bufs=2, space="PSUM") as ps0d:
            for b in range(BL):
                for q in range(4):
                    pm = ps0d.tile([128, 512], f32, tag="pmd")
                    for kt in range(4):
                        nc.tensor.matmul(
                            pm[:],
                            mtv[kt][:, 128 * b : 128 * (b + 1)],
                            wmodc[kt][:, 512 * q : 512 * (q + 1)],
                            start=(kt == 0),
                            stop=(kt == 3),
                        )
                    hd = mph[b][:, 512 * q : 512 * (q + 1)]
                    ld = mpl[b][:, 512 * q : 512 * (q + 1)]
                    nc.vector.tensor_copy(hd, pm[:])
                    nc.vector.tensor_tensor(ld, pm[:], hd, op=OP.subtract)
        wmodc_cm.__exit__(None, None, None)
        mtv_cm.__exit__(None, None, None)

        # ------------------------------------------------------------------
        # phase 1: the recurrence
        # ------------------------------------------------------------------
        with (
            tc.tile_pool(name="wk", bufs=1) as wk,
            tc.tile_pool(name="xwp", bufs=2) as xwp,
            tc.tile_pool(name="cst", bufs=2) as cst,
            tc.tile_pool(name="pz", bufs=5, space="PSUM") as pzp,
            tc.tile_pool(name="pat", bufs=2, space="PSUM") as patp,
            tc.tile_pool(name="ptr", bufs=1, space="PSUM") as ptrp,
        ):
            c2 = cst.tile([BL, U], f32, tag="c")
            nc.sync.dma_start(c2[:], enc_c_d[:])  # host passes 2*enc_c

            xwc = {}

            def load_xw_chunk(c):
                th_ = xwp.tile([128, G], f16, tag="xwh", name=f"xwh{c}")
                tl_ = xwp.tile([128, G], f16, tag="xwl", name=f"xwl{c}")
                rows = min(128, NTOK - 128 * c)
                nc.sync.dma_start(th_[:rows, :], xw_hi_dram[128 * c : 128 * c + rows])
                nc.sync.dma_start(tl_[:rows, :], xw_lo_dram[128 * c : 128 * c + rows])
                xwc[c] = (th_, tl_)

            load_xw_chunk(0)

            def hT_cols(tl, j, t0, ncols):
                v = tl[:].rearrange("p (j s) -> p j s", j=4)
                return v[:, j, 4 * t0 : 4 * t0 + ncols]

            def z_inject(t, pzq):
                """Xw hi/lo inject (+ t==1 corr): no dependency on h_{t-1};
                fills the PE while the previous step's gates evaluate."""
                ch = (t - 1) // 32
                row = 4 * ((t - 1) % 32)
                xh, xl = xwc[ch]
                for q in range(4):
                    zq = pzq[q][:]
                    nc.tensor.matmul(
                        zq, I128h[:, row : row + 4], xh[:, 512 * q : 512 * (q + 1)],
                        start=True, stop=False,
                    )
                    nc.tensor.matmul(
                        zq, I128h[:, row : row + 4], xl[:, 512 * q : 512 * (q + 1)],
                        start=False, stop=False,
                    )
                    if t == 1:
                        nc.tensor.matmul(
                            zq, I4n[:], corr[:, 512 * q : 512 * (q + 1)],
                            start=False, stop=False,
                        )

            def z_hpart(t, pzq, final):
                """h2_{t-1} @ Wmod_h, 3-pass fp16 hi/lo."""
                for kt in range(4):
                    hh = hT_cols(hth, kt, t - 1, 4)
                    hl = hT_cols(htl, kt, t - 1, 4)
                    for q in range(4):
                        zq = pzq[q][:]
                        nc.tensor.matmul(
                            zq, hh, wmh[kt][:, 512 * q : 512 * (q + 1)],
                            start=False, stop=False,
                        )
                        nc.tensor.matmul(
                            zq, hh, wml[kt][:, 512 * q : 512 * (q + 1)],
                            start=False, stop=False,
                        )
                        nc.tensor.matmul(
                            zq, hl, wmh[kt][:, 512 * q : 512 * (q + 1)],
                            start=False, stop=(final and kt == 3),
                        )

            def z_tail(t, pzq):
                """ctx contribution via alpha_{t-1} @ Mproj[b], 3-pass."""
                ec = 16 * (t - 2)
                for b in range(BL):
                    eh = eTh[:, ec + 4 * b : ec + 4 * b + 4]
                    el = eTl[:, ec + 4 * b : ec + 4 * b + 4]
                    for q in range(4):
                        zq = pzq[q][:]
                        nc.tensor.matmul(
                            zq, eh, mph[b][:, 512 * q : 512 * (q + 1)],
                            start=False, stop=False,
                        )
                        nc.tensor.matmul(
                            zq, eh, mpl[b][:, 512 * q : 512 * (q + 1)],
                            start=False, stop=False,
                        )
                        nc.tensor.matmul(
                            zq, el, mph[b][:, 512 * q : 512 * (q + 1)],
                            start=False, stop=(b == 3),
                        )

            def new_step_tiles(t):
                pzq = [
                    pzp.tile([BL, 512], f32, tag="pzq", name=f"pz{t}_{q}")
                    for q in range(4)
                ]
                psc = patp.tile([BL, BL * T], f32, tag="pat", name=f"psc{t}")
                return pzq, psc

            pzq_cur, psc_cur = new_step_tiles(1)
            z_inject(1, pzq_cur)
            nc.tensor.matmul(psc_cur[:], I4b[:], mneg[:], start=True, stop=False)
            z_hpart(1, pzq_cur, final=True)

            for t in range(1, n_steps + 1):
                if t % 32 == 2 and (t - 1) // 32 + 1 < n_chunks:
                    load_xw_chunk((t - 1) // 32 + 1)

                pzq, psc = pzq_cur, psc_cur

                # --- gates: per-q tanh chunks (i,f,g,o); f first ---
                th = wk.tile([BL, G], f32, tag="th")
                for q in (1, 0, 2, 3):
                    nc.scalar.activation(
                        th[:, 512 * q : 512 * (q + 1)], pzq[q][:],
                        AF.Tanh, bias=0.0, scale=0.5,
                    )

                # pre-issue t+1 PE work with no h_t dependency
                if t < n_steps:
                    pzq_cur, psc_cur = new_step_tiles(t + 1)
                    z_inject(t + 1, pzq_cur)
                    nc.tensor.matmul(
                        psc_cur[:], I4b[:], mneg[:], start=True, stop=False
                    )

                # --- cell update in the 2x basis ---
                # c2' = 0.5*(thf+1)*c2 + (thi+1)*tg ; h2 = (tho+1)*tanh(c2'/2)
                u4 = wk.tile([BL, U], f32, tag="u4")
                nc.vector.scalar_tensor_tensor(
                    u4[:], th[:, 512:1024], 1.0, c2[:], op0=OP.add, op1=OP.mult
                )
                v = wk.tile([BL, U], f32, tag="v")
                nc.vector.scalar_tensor_tensor(
                    v[:], th[:, 0:512], 1.0, th[:, 1024:1536],
                    op0=OP.add, op1=OP.mult,
                )
                c2n = cst.tile([BL, U], f32, tag="c")
                nc.vector.scalar_tensor_tensor(
                    c2n[:], u4[:], 0.5, v[:], op0=OP.mult, op1=OP.add
                )
                tc_ = wk.tile([BL, U], f32, tag="tc")
                nc.scalar.activation(tc_[:], c2n[:], AF.Tanh, bias=0.0, scale=0.5)
                h2 = wk.tile([BL, U], f32, tag="h")
                nc.vector.scalar_tensor_tensor(
                    h2[:], th[:, 1536:2048], 1.0, tc_[:], op0=OP.add, op1=OP.mult
                )
                c2 = c2n

                # --- hT hi/lo via PE transposes ---
                pht = ptrp.tile([128, 16], f32, tag="ptr")
                for j in range(4):
                    nc.tensor.transpose(
                        pht[:, 4 * j : 4 * j + 4], h2[:, 128 * j : 128 * (j + 1)], I4[:]
                    )
                phtv = pht[:].rearrange("p (j b) -> p j b", j=4)
                hiv = hth[:].rearrange("p (j s) -> p j s", j=4)[:, :, 4 * t : 4 * t + 4]
                lov = htl[:].rearrange("p (j s) -> p j s", j=4)[:, :, 4 * t : 4 * t + 4]
                nc.vector.tensor_copy(hiv, phtv)
                nc.vector.tensor_tensor(lov, phtv, hiv, op=OP.subtract)

                # --- score pairs [b, (b', t')] (mask pre-injected) ---
                for kt in range(4):
                    hh = hT_cols(hth, kt, t, 4)
                    hl = hT_cols(htl, kt, t, 4)
                    nc.tensor.matmul(psc[:], hh, kTh[kt][:], start=False, stop=False)
                    nc.tensor.matmul(psc[:], hh, kTl[kt][:], start=False, stop=False)
                    nc.tensor.matmul(
                        psc[:], hl, kTh[kt][:], start=False, stop=(kt == 3)
                    )

                # --- z_{t+1} h-part: fills the PE while softmax runs ---
                if t < n_steps:
                    z_hpart(t + 1, pzq_cur, final=False)

                # --- masked softmax straight off PSUM ---
                nmax = wk.tile([BL, 1], f32, tag="nmax")
                nc.vector.tensor_reduce(
                    nmax[:], psc[:], axis=AX.X, op=OP.max, negate=True
                )
                e = wk.tile([BL, BL * T], f32, tag="e")
                ssum = wk.tile([BL, 1], f32, tag="ssum")
                nc.scalar.activation(
                    e[:], psc[:], AF.Exp, bias=nmax[:, :1], scale=1.0,
                    accum_out=ssum[:, :1],
                )
                rec = wk.tile([BL, 1], f32, tag="rec")
                nc.vector.reciprocal(rec[:], ssum[:])
                e2 = wk.tile([BL, BL * T], f32, tag="e2")
                nc.vector.tensor_scalar(
                    e2[:], e[:], rec[:, :1], None, op0=OP.mult
                )

                # --- eT hi/lo blocks ---
                pet = ptrp.tile([128, 16], f32, tag="ptr")
                for q in range(BL):
                    nc.tensor.transpose(
                        pet[:, 4 * q : 4 * q + 4], e2[:, T * q : T * (q + 1)], I4[:]
                    )
                ehv = eTh[:, 16 * (t - 1) : 16 * t]
                elv = eTl[:, 16 * (t - 1) : 16 * t]
                nc.vector.tensor_copy(ehv, pet[:])
                nc.vector.tensor_tensor(elv, pet[:], ehv, op=OP.subtract)

                # --- z_{t+1} ctx part: needs eT of step t ---
                if t < n_steps:
                    z_tail(t + 1, pzq_cur)

        mproj_cm.__exit__(None, None, None)

        # ------------------------------------------------------------------
        # phase 2: ctxT from eT; attn = [H|CTX] @ Wa (fp16); logits = attn @ Wfc
        # ------------------------------------------------------------------
        with (
            tc.tile_pool(name="p2", bufs=1) as p2,
            tc.tile_pool(name="p2r", bufs=3) as p2r,
        ):
            ntok = BL * n_steps

            ps2a_cm = tc.tile_pool(name="ps2a", bufs=2, space="PSUM")
            ps2 = ps2a_cm.__enter__()

            # ctxT: ct2[j][:, 4*k + b] = ctx_{k+1}[b, 128j + u']
            ct2 = [p2.tile([128, NTOK], f16, tag=f"ct2{j}", name=f"ct2{j}") for j in range(4)]
            eview = eTh[:].rearrange("p (t s) -> p t s", s=16)
            for j in range(4):
                for b in range(BL):
                    pc2 = ps2.tile([128, T], f32, tag="pc2", bufs=2)
                    nc.tensor.matmul(
                        pc2[:, :n_steps],
                        mpack[:, U * b + 128 * j : U * b + 128 * (j + 1)],
                        eview[:, 0:n_steps, 4 * b + b],
                        start=True,
                        stop=True,
                    )
                    dst = ct2[j][:].rearrange("p (k b) -> p k b", b=4)[
                        :, 0:n_steps, b
                    ]
                    nc.vector.tensor_copy(dst, pc2[:, :n_steps])

            waxf = [p2.tile([128, U], f32, tag=f"waxf{k}", name=f"waxf{k}") for k in range(8)]
            wax = [p2.tile([128, U], f16, tag=f"wax{k}", name=f"wax{k}") for k in range(8)]
            for k in range(8):
                nc.sync.dma_start(waxf[k][:], Wa_d[128 * k : 128 * (k + 1)])
                nc.vector.tensor_copy(wax[k][:], waxf[k][:])
            att = [p2.tile([128, NTOK], fr, tag=f"att{j}", name=f"att{j}") for j in range(4)]
            for j in range(4):
                pa = ps2.tile([128, 512], f32, tag="pa", bufs=2)
                for kt in range(8):
                    if kt < 4:
                        src = hth[:].rearrange("p (jj s) -> p jj s", jj=4)[
                            :, kt, 4 : 4 + ntok
                        ]
                    else:
                        src = ct2[kt - 4][:, :ntok]
                    nc.tensor.matmul(
                        pa[:, :ntok],
                        wax[kt][:, 128 * j : 128 * (j + 1)],
                        src,
                        start=(kt == 0),
                        stop=(kt == 7),
                    )
                nc.vector.tensor_copy(att[j][:, :ntok], pa[:, :ntok])

            ps2a_cm.__exit__(None, None, None)
            ps2b_cm = tc.tile_pool(name="ps2b", bufs=2, space="PSUM")
            ps2 = ps2b_cm.__enter__()

            NCH = (VO + 511) // 512  # 63
            for nci in range(NCH):
                # last chunk overlaps the previous one so every chunk is a
                # full 512 wide (fp32r matmul needs aligned free dims)
                n0 = min(512 * nci, VO - 512)
                ncols = 512
                wff = p2r.tile([128, 4, 512], f32, tag="wff", bufs=4)
                nc.sync.dma_start(
                    wff[:, :, :ncols],
                    Wfc_d[:, n0 : n0 + ncols].rearrange("(k p) n -> p k n", k=4),
                )
                # round-produce f32r on the (idle) ACT engine: fast HW DMA path
                # for the 65MB Wfc stream instead of the software cast-DMA
                wf = p2r.tile([128, 4, 512], fr, tag="wf")
                nc.scalar.activation(
                    wf[:].rearrange("p k n -> p (k n)"),
                    wff[:].rearrange("p k n -> p (k n)"),
                    AF.Copy, bias=0.0, scale=1.0,
                )
                bfc_t = p2r.tile([1, 512], f32, tag="bfc")
                nc.sync.dma_start(bfc_t[:1, :ncols], bfc_d[:1, n0 : n0 + ncols])
                # broadcast bias across partitions once per chunk (K=1 matmul)
                pbc = ps2.tile([128, 512], f32, tag="pbc", bufs=2)
                nc.tensor.matmul(
                    pbc[:, :ncols], ones1[:1, :], bfc_t[:1, :ncols],
                    start=True, stop=True,
                )
                bfcs = p2r.tile([128, 512], f32, tag="bfcs")
                nc.vector.tensor_copy(bfcs[:, :ncols], pbc[:, :ncols])
                for mt in range((ntok + 127) // 128):
                    mrows = min(128, ntok - 128 * mt)
                    pl = ps2.tile([128, 512], f32, tag="pl", bufs=5)
                    for kt in range(4):
                        nc.tensor.matmul(
                            pl[:mrows, :ncols],
                            att[kt][:, 128 * mt : 128 * mt + mrows],
                            wf[:, kt, :ncols],
                            start=(kt == 0),
                            stop=(kt == 3),
                        )
                    # bias folded into the PSUM->SBUF copy (per-column bcast add)
                    ot = p2r.tile([128, 512], f32, tag="ot")
                    nc.vector.scalar_tensor_tensor(
                        ot[:mrows, :ncols], pl[:mrows, :ncols], 1.0,
                        bfcs[:mrows, :ncols], op0=OP.mult, op1=OP.add,
                    )
                    nc.scalar.dma_start(
                        out_d[128 * mt : 128 * mt + mrows, n0 : n0 + ncols],
                        ot[:mrows, :ncols],
                    )

            ps2b_cm.__exit__(None, None, None)

        dram_cm.__exit__(None, None, None)
        per_cm.__exit__(None, None, None)

    nc.compile()
    return nc


def _shard_inputs(inputs, memory, enc_h, enc_c, E, Wm, W_lstm, U_lstm, b_lstm, Wa, Wfc, bfc):
    inputs = np.ascontiguousarray(inputs)
    # h2 = 2h convention: pre-halve everything h multiplies, double the carries
    Wa_mod = np.concatenate([0.5 * Wa[:U], Wa[U:]], axis=0)
    shared = {
        "E": np.ascontiguousarray(E, np.float32),
        "Wm": np.ascontiguousarray(0.5 * Wm, np.float32),
        "W1": np.ascontiguousarray(W_lstm[:D], np.float32),
        "W2": np.ascontiguousarray(W_lstm[D:], np.float32),
        "Ul": np.ascontiguousarray(0.5 * U_lstm, np.float32),
        "bl": np.ascontiguousarray(b_lstm.reshape(1, G), np.float32),
        "Wa": np.ascontiguousarray(Wa_mod, np.float32),
        "Wfc": np.ascontiguousarray(Wfc, np.float32),
        "bfc": np.ascontiguousarray(bfc.reshape(1, VO), np.float32),
    }
    in_maps = []
    for rk in range(NCORES):
        sl = slice(BL * rk, BL * (rk + 1))
        m = dict(shared)
        m["tok_ids"] = np.ascontiguousarray(
            inputs[sl].T.reshape(NTOK, 1), np.int32
        )
        m["mem"] = np.ascontiguousarray(memory[sl], np.float32)
        m["enc_ht"] = np.ascontiguousarray(2.0 * enc_h[sl].T, np.float32)
        m["enc_c"] = np.ascontiguousarray(2.0 * enc_c[sl], np.float32)
        in_maps.append(m)
    return in_maps


def kernel(**inputs):
    from concourse.bass_utils import run_bass_kernel_spmd

    if "nc" not in _cache:
        _cache["nc"] = _build(T)
    nc = _cache["nc"]

    in_maps = _shard_inputs(**inputs)
    res = run_bass_kernel_spmd(nc, in_maps, core_ids=list(range(NCORES)))
    outs = []
    for rk in range(NCORES):
        o = res.results[rk]["out"]  # [512, 32001], rows (t, b)
        outs.append(o.reshape(T, BL, VO).transpose(1, 0, 2))
    full = np.concatenate(outs, axis=0)  # [32, 128, 32001]
    return full.astype(np.float32)


# revision 31
# speedup vs baseline: 1.2008x; 1.0324x over previous
"""Trainium2 Bass kernel for nn_Decoder (LSTM decoder + Luong attention + vocab proj).

Strategy (8 cores, data-parallel over batch, B_local = 4):
  phase 0: on-device prep per core:
    - embedding gather (indirect DMA) + Xw = X @ W1 + b precomputed for all steps,
      stored in DRAM as fp16 hi/lo pairs [512 tok, 2048].
    - keysT = (memory @ Wm')^T per batch -> fp16 hi/lo (Wm' = Wm/2, h2 convention)
    - fold attention out-proj into the recurrence:
        Wmod_h = Ul' + Wa_h' @ W2   (host pre-scales Ul, Wa_h by 1/2)
        Wmod_c = Wa_c @ W2          (g-gate cols pre-scaled x2)
      stored as fp16 hi/lo pairs.
    - Mproj[b] = mem[b] @ Wmod_c -> fp16 hi/lo.
    - step-1 correction corr = h_0 @ (Wa_h' @ W2) (since attn_0 = 0), fp32.
  phase 1: 128 sequential steps. All recurrence GEMMs run as 3-pass fp16
    hi/lo compensated matmuls (x_hi@w_hi + x_hi@w_lo + x_lo@w_hi): fp32-grade
    accuracy (measured 3.6e-7 per GEMM on HW) at bf16 PE rate (1 cyc/row vs
    4 for fp32). The cell state carries c2 = 2c and h2 = 2h so the sigmoid
    transform folds into fused scalar_tensor_tensor ops:
        c2' = 0.5*(tanh(zf/2)+1)*c2 + (tanh(zi/2)+1)*tanh(zg)
        h2  = (tanh(zo/2)+1)*tanh(c2'/2)
    z PSUM is split into 4 per-gate bank tiles so next step's Xw inject +
    score mask run on the PE while the current step's gates evaluate.
  phase 2: ctx materialized in batch from eT (fp16), attn = [H|CTX] @ Wa as
    fp16 GEMM, then logits = attn @ Wfc streaming Wfc fp32 via fast HW DMA and
    round-producing f32r tiles on the ACT engine; out rows are (t, b) tokens.
"""

import sys

for _p in ("/opt/trn_rl_repo",):
    if _p not in sys.path:
        sys.path.insert(0, _p)

import numpy as np

B, T, V, D, U = 32, 128, 32000, 256, 512
VO = V + 1
NCORES = 8
BL = B // NCORES  # 4
G = 4 * U  # 2048
NTOK = BL * T  # 512 tokens per core
HT_W = 4 * (T + 1)  # 516 columns per u-chunk in hT buffer

_cache = {}


def _build(n_steps=T):
    import concourse.bacc as bacc
    import concourse.bass as bass
    import concourse.mybir as mybir
    import concourse.tile as tile
    from concourse.masks import make_identity

    f32 = mybir.dt.float32
    f16 = mybir.dt.float16
    fr = mybir.dt.float32r  # full-rate PE path: phase-2 only (error hits logits directly)
    bf = mybir.dt.bfloat16
    i32 = mybir.dt.int32
    AX = mybir.AxisListType
    OP = mybir.AluOpType
    AF = mybir.ActivationFunctionType

    try:
        import concourse.tile_utils as _tu

        if getattr(_tu, "max_sbuf_usage", 0) < 204 * 1024:
            _tu.max_sbuf_usage = 204 * 1024
    except Exception:
        pass

    nc = bacc.Bacc(None, target_bir_lowering=False)

    tok_ids = nc.dram_tensor("tok_ids", [NTOK, 1], i32, kind="ExternalInput")
    mem_d = nc.dram_tensor("mem", [BL, T, U], f32, kind="ExternalInput")
    enc_ht_d = nc.dram_tensor("enc_ht", [U, BL], f32, kind="ExternalInput")  # 2*enc_h^T
    enc_c_d = nc.dram_tensor("enc_c", [BL, U], f32, kind="ExternalInput")   # 2*enc_c
    E_d = nc.dram_tensor("E", [V, D], f32, kind="ExternalInput")
    Wm_d = nc.dram_tensor("Wm", [U, U], f32, kind="ExternalInput")          # Wm/2
    W1_d = nc.dram_tensor("W1", [D, G], f32, kind="ExternalInput")
    W2_d = nc.dram_tensor("W2", [U, G], f32, kind="ExternalInput")
    Ul_d = nc.dram_tensor("Ul", [U, G], f32, kind="ExternalInput")          # Ul/2
    bl_d = nc.dram_tensor("bl", [1, G], f32, kind="ExternalInput")
    Wa_d = nc.dram_tensor("Wa", [2 * U, U], f32, kind="ExternalInput")      # [Wa_h/2; Wa_c]
    Wfc_d = nc.dram_tensor("Wfc", [U, VO], f32, kind="ExternalInput")
    bfc_d = nc.dram_tensor("bfc", [1, VO], f32, kind="ExternalInput")
    out_d = nc.dram_tensor("out", [NTOK, VO], f32, kind="ExternalOutput")

    n_chunks = (n_steps * BL + 127) // 128

    with tile.TileContext(nc) as tc:
        # ------------------------------------------------------------------
        # persistent pool
        # ------------------------------------------------------------------
        per_cm = tc.tile_pool(name="per", bufs=1)
        per = per_cm.__enter__()
        dram_cm = tc.tile_pool(name="dram", bufs=1, space="DRAM")
        dram = dram_cm.__enter__()

        wmh = [per.tile([128, G], f16, tag=f"wmh{k}", name=f"wmh{k}") for k in range(4)]
        wml = [per.tile([128, G], f16, tag=f"wml{k}", name=f"wml{k}") for k in range(4)]
        kTh = [per.tile([128, BL * T], f16, tag=f"kTh{j}", name=f"kTh{j}") for j in range(4)]
        kTl = [per.tile([128, BL * T], f16, tag=f"kTl{j}", name=f"kTl{j}") for j in range(4)]
        mpack = per.tile([128, BL * U], f16, tag="mpack")  # [t, (b,u)]; phase-2 only
        hth = per.tile([128, 4 * HT_W], f16, tag="hth")
        htl = per.tile([128, 4 * HT_W], f16, tag="htl")
        eTh = per.tile([128, 16 * T], f16, tag="eTh")
        eTl = per.tile([128, 16 * T], f16, tag="eTl")
        corr = per.tile([BL, G], f32, tag="corr")
        I4 = per.tile([4, 4], f32, tag="I4")        # f32: transpose identity
        I4b = per.tile([4, 4], bf, tag="I4b")       # bf16 lhsT for the mask matmul
        I4n = per.tile([4, 4], f32, tag="I4n")      # -I: corr inject (fp32)
        I128 = per.tile([128, 128], f32, tag="I128")
        I128h = per.tile([128, 128], f16, tag="I128h")
        ones1 = per.tile([1, 128], f32, tag="ones1")
        mneg = per.tile([BL, BL * T], bf, tag="mneg")
        mnegf = per.tile([BL, BL * T], f32, tag="mnegf")

        make_identity(nc, I4[:])
        make_identity(nc, I128[:])
        nc.vector.tensor_copy(I4b[:], I4[:])
        nc.vector.tensor_scalar_mul(I4n[:], I4[:], -1.0)
        nc.vector.tensor_copy(I128h[:], I128[:])
        onesf = per.tile([1, 128], f32, tag="onesf")
        nc.gpsimd.memset(onesf[:], 1.0)
        nc.vector.tensor_copy(ones1[:], onesf[:])
        ones16 = per.tile([1, 128], f16, tag="ones16")
        nc.vector.tensor_copy(ones16[:], onesf[:])
        # block-diagonal additive mask: 0 on own 128-block, -1e30 elsewhere.
        miot = per.tile([BL, BL * T], f32, tag="miot")
        nc.gpsimd.iota(
            miot[:], pattern=[[1, BL * T]], base=0, channel_multiplier=-T,
            allow_small_or_imprecise_dtypes=True,
        )
        ma = per.tile([BL, BL * T], f32, tag="ma")
        nc.vector.tensor_scalar(ma[:], miot[:], 0.0, None, op0=OP.is_ge)
        nc.vector.tensor_scalar(mnegf[:], miot[:], float(T - 1), None, op0=OP.is_le)
        nc.vector.tensor_tensor(ma[:], ma[:], mnegf[:], op=OP.mult)
        nc.vector.tensor_scalar(mneg[:], ma[:], -1.0, 1e30, op0=OP.add, op1=OP.mult)

        xw_hi_dram = dram.tile([NTOK, G], f16, name="xw_hi_dram")
        xw_lo_dram = dram.tile([NTOK, G], f16, name="xw_lo_dram")
        wfc16_dram = dram.tile([U, VO], f16, name="wfc16_dram")

        # ------------------------------------------------------------------
        # phase 0a: embedding gather + Xw = X @ W1 + bl (g cols x2) -> fp16
        # hi/lo in DRAM; memT (+ mpack fp16); keysT -> fp16 hi/lo
        # ------------------------------------------------------------------
        mproj_cm = tc.tile_pool(name="mprojp", bufs=1)
        mprojp = mproj_cm.__enter__()
        mph = [mprojp.tile([128, G], f16, tag=f"mph{b}", name=f"mph{b}") for b in range(BL)]
        mpl = [mprojp.tile([128, G], f16, tag=f"mpl{b}", name=f"mpl{b}") for b in range(BL)]
        mtv_cm = tc.tile_pool(name="mtvp", bufs=1)
        mtvp = mtv_cm.__enter__()
        mtv = [mtvp.tile([128, BL * 128], f32, tag=f"mtv{v}", name=f"mtv{v}") for v in range(4)]
        wmodc_cm = tc.tile_pool(name="wmodcp", bufs=1)
        wmodcp = wmodc_cm.__enter__()
        wmodc = [wmodcp.tile([128, G], f32, tag=f"wmodc{k}", name=f"wmodc{k}") for k in range(4)]

        with (
            tc.tile_pool(name="p0a", bufs=2) as p0a,
            tc.tile_pool(name="p0a1", bufs=1) as p0a1,
            tc.tile_pool(name="ps0", bufs=2, space="PSUM") as ps0,
        ):
            # init h2_0 = 2*enc_h (host-prescaled), fp16 hi/lo
            h0f = p0a1.tile([128, 4, BL], f32, tag="h0f")
            nc.sync.dma_start(h0f[:], enc_ht_d[:].rearrange("(j p) b -> p j b", j=4))
            h0hi = hth[:].rearrange("p (j s) -> p j s", j=4)[:, :, 0:BL]
            h0lo = htl[:].rearrange("p (j s) -> p j s", j=4)[:, :, 0:BL]
            nc.vector.tensor_copy(h0hi, h0f[:])
            nc.vector.tensor_tensor(h0lo, h0f[:], h0hi, op=OP.subtract)

            bls = p0a1.tile([1, G], f32, tag="bls")
            nc.sync.dma_start(bls[:], bl_d[:])
            # broadcast bl across partitions once (g cols x2 for the tanh trick)
            blsb = p0a1.tile([128, G], f32, tag="blsb")
            for q in range(4):
                pbl = ps0.tile([128, 512], f32, tag="pbl")
                nc.tensor.matmul(
                    pbl[:], ones1[:1, :], bls[:1, 512 * q : 512 * (q + 1)],
                    start=True, stop=True,
                )
                if q == 2:
                    nc.vector.tensor_scalar_mul(
                        blsb[:, 512 * q : 512 * (q + 1)], pbl[:], 2.0
                    )
                else:
                    nc.vector.tensor_copy(blsb[:, 512 * q : 512 * (q + 1)], pbl[:])
            xt = [p0a1.tile([128, NTOK], f32, tag=f"xt{k}", name=f"xt{k}") for k in range(2)]

            for c in range(NTOK // 128):
                ids_c = p0a.tile([128, 1], i32, tag="ids")
                nc.sync.dma_start(ids_c[:], tok_ids[128 * c : 128 * (c + 1)])
                x_c = p0a.tile([128, D], f32, tag="xc")
                nc.gpsimd.indirect_dma_start(
                    out=x_c[:],
                    out_offset=None,
                    in_=E_d[:],
                    in_offset=bass.IndirectOffsetOnAxis(ap=ids_c[:, :1], axis=0),
                )
                for k in range(2):
                    pt = ps0.tile([128, 128], f32, tag="pt0")
                    nc.tensor.transpose(pt[:], x_c[:, 128 * k : 128 * (k + 1)], I128[:])
                    nc.vector.tensor_copy(xt[k][:, 128 * c : 128 * (c + 1)], pt[:])

            for q in range(4):
                w1q = [
                    p0a.tile([128, 512], f32, tag="w1q", name=f"w1q{q}_{k}")
                    for k in range(2)
                ]
                for k in range(2):
                    nc.sync.dma_start(
                        w1q[k][:],
                        W1_d[128 * k : 128 * (k + 1), 512 * q : 512 * (q + 1)],
                    )
                for c in range(NTOK // 128):
                    pz0 = ps0.tile([128, 512], f32, tag="pz0")
                    for k in range(2):
                        nc.tensor.matmul(
                            pz0[:],
                            xt[k][:, 128 * c : 128 * (c + 1)],
                            w1q[k][:],
                            start=(k == 0),
                            stop=(k == 1),
                        )
                    st = p0a.tile([128, 512], f32, tag="xwst")
                    nc.vector.scalar_tensor_tensor(
                        st[:], pz0[:], 2.0 if q == 2 else 1.0,
                        blsb[:, 512 * q : 512 * (q + 1)],
                        op0=OP.mult, op1=OP.add,
                    )
                    sh = p0a.tile([128, 512], f16, tag="xwsh")
                    sl = p0a.tile([128, 512], f16, tag="xwsl")
                    nc.vector.tensor_copy(sh[:], st[:])
                    nc.vector.tensor_tensor(sl[:], st[:], sh[:], op=OP.subtract)
                    nc.sync.dma_start(
                        xw_hi_dram[128 * c : 128 * (c + 1), 512 * q : 512 * (q + 1)],
                        sh[:],
                    )
                    nc.sync.dma_start(
                        xw_lo_dram[128 * c : 128 * (c + 1), 512 * q : 512 * (q + 1)],
                        sl[:],
                    )

            # memT: mtv[vc][:, 128*b + t] = mem[b, t, 128*vc + v']; mpack fp16
            for b in range(BL):
                memf = p0a.tile([128, U], f32, tag="memf", name=f"memf{b}")
                nc.sync.dma_start(memf[:], mem_d[b])
                nc.vector.tensor_copy(mpack[:, U * b : U * (b + 1)], memf[:])
                for vc in range(4):
                    pt = ps0.tile([128, 128], f32, tag="pt0")
                    nc.tensor.transpose(
                        pt[:], memf[:, 128 * vc : 128 * (vc + 1)], I128[:]
                    )
                    nc.vector.tensor_copy(mtv[vc][:, 128 * b : 128 * (b + 1)], pt[:])

            # keysT (Wm pre-halved on host for the h2 convention) -> fp16 hi/lo
            wms = [p0a1.tile([128, U], f32, tag=f"wms{k}", name=f"wms{k}") for k in range(4)]
            for k in range(4):
                nc.sync.dma_start(wms[k][:], Wm_d[128 * k : 128 * (k + 1)])
            for j in range(4):
                for b in range(BL):
                    pk = ps0.tile([128, 128], f32, tag="pt0")
                    for vt in range(4):
                        nc.tensor.matmul(
                            pk[:],
                            wms[vt][:, 128 * j : 128 * (j + 1)],
                            mtv[vt][:, 128 * b : 128 * (b + 1)],
                            start=(vt == 0),
                            stop=(vt == 3),
                        )
                    hd = kTh[j][:, 128 * b : 128 * (b + 1)]
                    ld = kTl[j][:, 128 * b : 128 * (b + 1)]
                    nc.vector.tensor_copy(hd, pk[:])
                    nc.vector.tensor_tensor(ld, pk[:], hd, op=OP.subtract)

        # ------------------------------------------------------------------
        # phase 0c: Wmod_h = Ul' + Wa_h' @ W2 -> fp16 hi/lo (g cols x2);
        #           Wmod_c = Wa_c @ W2 (f32, feeds Mproj); corr
        # ------------------------------------------------------------------
        with (
            tc.tile_pool(name="p0c", bufs=1) as p0c,
            tc.tile_pool(name="p0cr", bufs=2) as p0cr,
            tc.tile_pool(name="p0w2", bufs=4) as p0w2,
            tc.tile_pool(name="ps0c", bufs=2, space="PSUM") as ps0c,
        ):
            was = [p0c.tile([128, U], f32, tag=f"was{k}", name=f"was{k}") for k in range(8)]
            for k in range(8):
                nc.sync.dma_start(was[k][:], Wa_d[128 * k : 128 * (k + 1)])
            wat = [p0c.tile([128, 2 * U], f32, tag=f"wat{q}", name=f"wat{q}") for q in range(4)]
            for k in range(8):
                for q in range(4):
                    pt = ps0c.tile([128, 128], f32, tag="ptc")
                    nc.tensor.transpose(
                        pt[:], was[k][:, 128 * q : 128 * (q + 1)], I128[:]
                    )
                    nc.vector.tensor_copy(wat[q][:, 128 * k : 128 * (k + 1)], pt[:])

            # enc_ht (=2*enc_h^T) as lhsT tiles: ehts[:, 4*kt + b]
            ehts = p0c.tile([128, 16], f32, tag="ehts")
            nc.sync.dma_start(
                ehts[:].rearrange("p (k b) -> p k b", k=4),
                enc_ht_d[:].rearrange("(k p) b -> p k b", k=4),
            )

            # corr: s = h2_0 @ Wa_h' = h_0 @ Wa_h ; corr = s @ W2 (g cols x2)
            ps_s = ps0c.tile([4, 512], f32, tag="ps_s")
            for kt in range(4):
                nc.tensor.matmul(
                    ps_s[:],
                    ehts[:, 4 * kt : 4 * kt + 4],
                    was[kt][:],
                    start=(kt == 0),
                    stop=(kt == 3),
                )
            s_sb = p0c.tile([4, 512], f32, tag="s_sb")
            nc.vector.tensor_copy(s_sb[:], ps_s[:])
            stT = p0c.tile([128, 16], f32, tag="stT")
            for j in range(4):
                pt = ps0c.tile([128, 16], f32, tag="pts")
                nc.tensor.transpose(
                    pt[:, 4 * j : 4 * j + 4], s_sb[:, 128 * j : 128 * (j + 1)], I4[:]
                )
                nc.vector.tensor_copy(stT[:, 4 * j : 4 * j + 4], pt[:, 4 * j : 4 * j + 4])

            # Mfold rows chunk mc (q-outer so W2 slices are loaded once)
            for q in range(4):
                w2q = [
                    p0w2.tile([128, 512], f32, tag="w2q", name=f"w2q{q}_{kt}")
                    for kt in range(4)
                ]
                for kt in range(4):
                    nc.sync.dma_start(
                        w2q[kt][:],
                        W2_d[128 * kt : 128 * (kt + 1), 512 * q : 512 * (q + 1)],
                    )
                for mc in range(8):
                    pm = ps0c.tile([128, 512], f32, tag="pm")
                    for kt in range(4):
                        nc.tensor.matmul(
                            pm[:],
                            wat[kt][:, 128 * mc : 128 * (mc + 1)],
                            w2q[kt][:],
                            start=(kt == 0),
                            stop=(kt == 3),
                        )
                    scl = 2.0 if q == 2 else 1.0
                    if mc < 4:
                        # h rows: Ul' chunk + Mfold (then g-scale) -> fp16 hi/lo
                        ul_t = p0cr.tile([128, 512], f32, tag="ul")
                        nc.sync.dma_start(
                            ul_t[:],
                            Ul_d[128 * mc : 128 * (mc + 1), 512 * q : 512 * (q + 1)],
                        )
                        sc = p0cr.tile([128, 512], f32, tag="sc")
                        if q == 2:
                            tmp = p0cr.tile([128, 512], f32, tag="gtmp")
                            nc.vector.tensor_tensor(tmp[:], pm[:], ul_t[:], op=OP.add)
                            nc.vector.tensor_scalar_mul(sc[:], tmp[:], 2.0)
                        else:
                            nc.vector.tensor_tensor(sc[:], pm[:], ul_t[:], op=OP.add)
                        hd = wmh[mc][:, 512 * q : 512 * (q + 1)]
                        ld = wml[mc][:, 512 * q : 512 * (q + 1)]
                        nc.vector.tensor_copy(hd, sc[:])
                        nc.vector.tensor_tensor(ld, sc[:], hd, op=OP.subtract)
                    else:
                        dst = wmodc[mc - 4][:, 512 * q : 512 * (q + 1)]
                        nc.scalar.activation(dst, pm[:], AF.Copy, bias=0.0, scale=scl)

                # corr chunk q while w2q is resident
                pc = ps0c.tile([4, 512], f32, tag="ps_s")
                for kt in range(4):
                    nc.tensor.matmul(
                        pc[:],
                        stT[:, 4 * kt : 4 * kt + 4],
                        w2q[kt][:],
                        start=(kt == 0),
                        stop=(kt == 3),
                    )
                nc.scalar.activation(
                    corr[:, 512 * q : 512 * (q + 1)],
                    pc[:],
                    AF.Copy,
                    bias=0.0,
                    scale=2.0 if q == 2 else 1.0,
                )

        # ------------------------------------------------------------------
        # phase 0d: Mproj[b] = mem[b] @ Wmod_c -> fp16 hi/lo
        # ------------------------------------------------------------------
        with tc.tile_pool(name="ps0d", bufs=2, space="PSUM") as ps0d:
            for b in range(BL):
                for q in range(4):
                    pm = ps0d.tile([128, 512], f32, tag="pmd")
                    for kt in range(4):
                        nc.tensor.matmul(
                            pm[:],
                            mtv[kt][:, 128 * b : 128 * (b + 1)],
                            wmodc[kt][:, 512 * q : 512 * (q + 1)],
                            start=(kt == 0),
                            stop=(kt == 3),
                        )
                    hd = mph[b][:, 512 * q : 512 * (q + 1)]
                    ld = mpl[b][:, 512 * q : 512 * (q + 1)]
                    nc.vector.tensor_copy(hd, pm[:])
                    nc.vector.tensor_tensor(ld, pm[:], hd, op=OP.subtract)
        wmodc_cm.__exit__(None, None, None)
        mtv_cm.__exit__(None, None, None)

        # ------------------------------------------------------------------
        # phase 1: the recurrence
        # ------------------------------------------------------------------
        NCH = (VO + 511) // 512  # 63
        with (
            tc.tile_pool(name="wk", bufs=1) as wk,
            tc.tile_pool(name="xwp", bufs=2) as xwp,
            tc.tile_pool(name="wcv", bufs=2) as wcv,
            tc.tile_pool(name="cst", bufs=2) as cst,
            tc.tile_pool(name="pz", bufs=5, space="PSUM") as pzp,
            tc.tile_pool(name="pat", bufs=2, space="PSUM") as patp,
            tc.tile_pool(name="ptr", bufs=1, space="PSUM") as ptrp,
        ):
            c2 = cst.tile([BL, U], f32, tag="c")
            nc.sync.dma_start(c2[:], enc_c_d[:])  # host passes 2*enc_c

            xwc = {}

            def load_xw_chunk(c):
                th_ = xwp.tile([128, G], f16, tag="xwh", name=f"xwh{c}")
                tl_ = xwp.tile([128, G], f16, tag="xwl", name=f"xwl{c}")
                rows = min(128, NTOK - 128 * c)
                nc.sync.dma_start(th_[:rows, :], xw_hi_dram[128 * c : 128 * c + rows])
                nc.sync.dma_start(tl_[:rows, :], xw_lo_dram[128 * c : 128 * c + rows])
                xwc[c] = (th_, tl_)

            load_xw_chunk(0)

            def hT_cols(tl, j, t0, ncols):
                v = tl[:].rearrange("p (j s) -> p j s", j=4)
                return v[:, j, 4 * t0 : 4 * t0 + ncols]

            def z_inject(t, pzq):
                """Xw hi/lo inject (+ t==1 corr): no dependency on h_{t-1};
                fills the PE while the previous step's gates evaluate."""
                ch = (t - 1) // 32
                row = 4 * ((t - 1) % 32)
                xh, xl = xwc[ch]
                for q in range(4):
                    zq = pzq[q][:]
                    nc.tensor.matmul(
                        zq, I128h[:, row : row + 4], xh[:, 512 * q : 512 * (q + 1)],
                        start=True, stop=False,
                    )
                    nc.tensor.matmul(
                        zq, I128h[:, row : row + 4], xl[:, 512 * q : 512 * (q + 1)],
                        start=False, stop=False,
                    )
                    if t == 1:
                        nc.tensor.matmul(
                            zq, I4n[:], corr[:, 512 * q : 512 * (q + 1)],
                            start=False, stop=False,
                        )

            def z_hpart(t, pzq, final):
                """h2_{t-1} @ Wmod_h, 3-pass fp16 hi/lo."""
                for kt in range(4):
                    hh = hT_cols(hth, kt, t - 1, 4)
                    hl = hT_cols(htl, kt, t - 1, 4)
                    for q in range(4):
                        zq = pzq[q][:]
                        nc.tensor.matmul(
                            zq, hh, wmh[kt][:, 512 * q : 512 * (q + 1)],
                            start=False, stop=False,
                        )
                        nc.tensor.matmul(
                            zq, hh, wml[kt][:, 512 * q : 512 * (q + 1)],
                            start=False, stop=False,
                        )
                        nc.tensor.matmul(
                            zq, hl, wmh[kt][:, 512 * q : 512 * (q + 1)],
                            start=False, stop=(final and kt == 3),
                        )

            def z_tail(t, pzq):
                """ctx contribution via alpha_{t-1} @ Mproj[b], 3-pass."""
                ec = 16 * (t - 2)
                for b in range(BL):
                    eh = eTh[:, ec + 4 * b : ec + 4 * b + 4]
                    el = eTl[:, ec + 4 * b : ec + 4 * b + 4]
                    for q in range(4):
                        zq = pzq[q][:]
                        nc.tensor.matmul(
                            zq, eh, mph[b][:, 512 * q : 512 * (q + 1)],
                            start=False, stop=False,
                        )
                        nc.tensor.matmul(
                            zq, eh, mpl[b][:, 512 * q : 512 * (q + 1)],
                            start=False, stop=False,
                        )
                        nc.tensor.matmul(
                            zq, el, mph[b][:, 512 * q : 512 * (q + 1)],
                            start=False, stop=(b == 3),
                        )

            def new_step_tiles(t):
                pzq = [
                    pzp.tile([BL, 512], f32, tag="pzq", name=f"pz{t}_{q}")
                    for q in range(4)
                ]
                psc = patp.tile([BL, BL * T], f32, tag="pat", name=f"psc{t}")
                return pzq, psc

            pzq_cur, psc_cur = new_step_tiles(1)
            z_inject(1, pzq_cur)
            nc.tensor.matmul(psc_cur[:], I4b[:], mneg[:], start=True, stop=False)
            z_hpart(1, pzq_cur, final=True)

            for t in range(1, n_steps + 1):
                if t % 32 == 2 and (t - 1) // 32 + 1 < n_chunks:
                    load_xw_chunk((t - 1) // 32 + 1)

                pzq, psc = pzq_cur, psc_cur

                # --- gates: per-q tanh chunks (i,f,g,o); f first ---
                th = wk.tile([BL, G], f32, tag="th")
                for q in (1, 0, 2, 3):
                    nc.scalar.activation(
                        th[:, 512 * q : 512 * (q + 1)], pzq[q][:],
                        AF.Tanh, bias=0.0, scale=0.5,
                    )

                # pre-issue t+1 PE work with no h_t dependency
                if t < n_steps:
                    pzq_cur, psc_cur = new_step_tiles(t + 1)
                    z_inject(t + 1, pzq_cur)
                    nc.tensor.matmul(
                        psc_cur[:], I4b[:], mneg[:], start=True, stop=False
                    )

                # --- cell update in the 2x basis ---
                # c2' = 0.5*(thf+1)*c2 + (thi+1)*tg ; h2 = (tho+1)*tanh(c2'/2)
                u4 = wk.tile([BL, U], f32, tag="u4")
                nc.vector.scalar_tensor_tensor(
                    u4[:], th[:, 512:1024], 1.0, c2[:], op0=OP.add, op1=OP.mult
                )
                v = wk.tile([BL, U], f32, tag="v")
                nc.vector.scalar_tensor_tensor(
                    v[:], th[:, 0:512], 1.0, th[:, 1024:1536],
                    op0=OP.add, op1=OP.mult,
                )
                c2n = cst.tile([BL, U], f32, tag="c")
                nc.vector.scalar_tensor_tensor(
                    c2n[:], u4[:], 0.5, v[:], op0=OP.mult, op1=OP.add
                )
                tc_ = wk.tile([BL, U], f32, tag="tc")
                nc.scalar.activation(tc_[:], c2n[:], AF.Tanh, bias=0.0, scale=0.5)
                h2 = wk.tile([BL, U], f32, tag="h")
                nc.vector.scalar_tensor_tensor(
                    h2[:], th[:, 1536:2048], 1.0, tc_[:], op0=OP.add, op1=OP.mult
                )
                c2 = c2n

                # --- hT hi/lo via PE transposes ---
                pht = ptrp.tile([128, 16], f32, tag="ptr")
                for j in range(4):
                    nc.tensor.transpose(
                        pht[:, 4 * j : 4 * j + 4], h2[:, 128 * j : 128 * (j + 1)], I4[:]
                    )
                phtv = pht[:].rearrange("p (j b) -> p j b", j=4)
                hiv = hth[:].rearrange("p (j s) -> p j s", j=4)[:, :, 4 * t : 4 * t + 4]
                lov = htl[:].rearrange("p (j s) -> p j s", j=4)[:, :, 4 * t : 4 * t + 4]
                nc.vector.tensor_copy(hiv, phtv)
                nc.vector.tensor_tensor(lov, phtv, hiv, op=OP.subtract)

                # --- score pairs [b, (b', t')] (mask pre-injected) ---
                for kt in range(4):
                    hh = hT_cols(hth, kt, t, 4)
                    hl = hT_cols(htl, kt, t, 4)
                    nc.tensor.matmul(psc[:], hh, kTh[kt][:], start=False, stop=False)
                    nc.tensor.matmul(psc[:], hh, kTl[kt][:], start=False, stop=False)
                    nc.tensor.matmul(
                        psc[:], hl, kTh[kt][:], start=False, stop=(kt == 3)
                    )

                # --- z_{t+1} h-part: fills the PE while softmax runs ---
                if t < n_steps:
                    z_hpart(t + 1, pzq_cur, final=False)

                # --- masked softmax straight off PSUM ---
                nmax = wk.tile([BL, 1], f32, tag="nmax")
                nc.vector.tensor_reduce(
                    nmax[:], psc[:], axis=AX.X, op=OP.max, negate=True
                )
                e = wk.tile([BL, BL * T], f32, tag="e")
                ssum = wk.tile([BL, 1], f32, tag="ssum")
                nc.scalar.activation(
                    e[:], psc[:], AF.Exp, bias=nmax[:, :1], scale=1.0,
                    accum_out=ssum[:, :1],
                )
                rec = wk.tile([BL, 1], f32, tag="rec")
                nc.vector.reciprocal(rec[:], ssum[:])
                e2 = wk.tile([BL, BL * T], f32, tag="e2")
                nc.vector.tensor_scalar(
                    e2[:], e[:], rec[:, :1], None, op0=OP.mult
                )

                # --- eT hi/lo blocks ---
                pet = ptrp.tile([128, 16], f32, tag="ptr")
                for q in range(BL):
                    nc.tensor.transpose(
                        pet[:, 4 * q : 4 * q + 4], e2[:, T * q : T * (q + 1)], I4[:]
                    )
                ehv = eTh[:, 16 * (t - 1) : 16 * t]
                elv = eTl[:, 16 * (t - 1) : 16 * t]
                nc.vector.tensor_copy(ehv, pet[:])
                nc.vector.tensor_tensor(elv, pet[:], ehv, op=OP.subtract)

                # --- z_{t+1} ctx part: needs eT of step t ---
                if t < n_steps:
                    z_tail(t + 1, pzq_cur)

                # trickle-convert Wfc to fp16 in DRAM while DMA is idle; the
                # ACT copy lands in its idle window after exp(t)
                kcv = (NCH + n_steps - 1) // n_steps
                for cc in range(kcv * (t - 1), min(kcv * t, NCH)):
                    n0c = min(512 * cc, VO - 512)
                    wcf = wcv.tile([128, 4, 512], f32, tag="wcf")
                    nc.sync.dma_start(
                        wcf[:],
                        Wfc_d[:, n0c : n0c + 512].rearrange("(k p) n -> p k n", k=4),
                    )
                    wc6 = wcv.tile([128, 4, 512], f16, tag="wc6")
                    nc.scalar.activation(
                        wc6[:].rearrange("p k n -> p (k n)"),
                        wcf[:].rearrange("p k n -> p (k n)"),
                        AF.Copy, bias=0.0, scale=1.0,
                    )
                    nc.sync.dma_start(
                        wfc16_dram[:, n0c : n0c + 512].rearrange(
                            "(k p) n -> p k n", k=4
                        ),
                        wc6[:],
                    )

        mproj_cm.__exit__(None, None, None)

        # ------------------------------------------------------------------
        # phase 2: ctxT from eT; attn = [H|CTX] @ Wa (fp16); logits = attn @ Wfc
        # ------------------------------------------------------------------
        with (
            tc.tile_pool(name="p2", bufs=1) as p2,
            tc.tile_pool(name="p2r", bufs=3) as p2r,
        ):
            ntok = BL * n_steps

            ps2a_cm = tc.tile_pool(name="ps2a", bufs=2, space="PSUM")
            ps2 = ps2a_cm.__enter__()

            # ctxT: ct2[j][:, 4*k + b] = ctx_{k+1}[b, 128j + u']
            ct2 = [p2.tile([128, NTOK], f16, tag=f"ct2{j}", name=f"ct2{j}") for j in range(4)]
            eview = eTh[:].rearrange("p (t s) -> p t s", s=16)
            for j in range(4):
                for b in range(BL):
                    pc2 = ps2.tile([128, T], f32, tag="pc2", bufs=2)
                    nc.tensor.matmul(
                        pc2[:, :n_steps],
                        mpack[:, U * b + 128 * j : U * b + 128 * (j + 1)],
                        eview[:, 0:n_steps, 4 * b + b],
                        start=True,
                        stop=True,
                    )
                    dst = ct2[j][:].rearrange("p (k b) -> p k b", b=4)[
                        :, 0:n_steps, b
                    ]
                    nc.vector.tensor_copy(dst, pc2[:, :n_steps])

            waxf = [p2.tile([128, U], f32, tag=f"waxf{k}", name=f"waxf{k}") for k in range(8)]
            wax = [p2.tile([128, U], f16, tag=f"wax{k}", name=f"wax{k}") for k in range(8)]
            for k in range(8):
                nc.sync.dma_start(waxf[k][:], Wa_d[128 * k : 128 * (k + 1)])
                nc.vector.tensor_copy(wax[k][:], waxf[k][:])
            att = [p2.tile([128, NTOK], f16, tag=f"att{j}", name=f"att{j}") for j in range(4)]
            for j in range(4):
                pa = ps2.tile([128, 512], f32, tag="pa", bufs=2)
                for kt in range(8):
                    if kt < 4:
                        src = hth[:].rearrange("p (jj s) -> p jj s", jj=4)[
                            :, kt, 4 : 4 + ntok
                        ]
                    else:
                        src = ct2[kt - 4][:, :ntok]
                    nc.tensor.matmul(
                        pa[:, :ntok],
                        wax[kt][:, 128 * j : 128 * (j + 1)],
                        src,
                        start=(kt == 0),
                        stop=(kt == 7),
                    )
                nc.vector.tensor_copy(att[j][:, :ntok], pa[:, :ntok])

            ps2a_cm.__exit__(None, None, None)
            ps2b_cm = tc.tile_pool(name="ps2b", bufs=2, space="PSUM")
            ps2 = ps2b_cm.__enter__()

            for nci in range(NCH):
                # last chunk overlaps the previous one so every chunk is a
                # full 512 wide (fp32r matmul needs aligned free dims)
                n0 = min(512 * nci, VO - 512)
                ncols = 512
                wf = p2r.tile([128, 4, 512], f16, tag="wf", bufs=4)
                nc.sync.dma_start(
                    wf[:, :, :ncols],
                    wfc16_dram[:, n0 : n0 + ncols].rearrange("(k p) n -> p k n", k=4),
                )
                bfc_t = p2r.tile([1, 512], f16, tag="bfc")
                nc.gpsimd.dma_start(bfc_t[:1, :ncols], bfc_d[:1, n0 : n0 + ncols])
                # broadcast bias across partitions once per chunk (K=1 matmul)
                pbc = ps2.tile([128, 512], f32, tag="pbc", bufs=2)
                nc.tensor.matmul(
                    pbc[:, :ncols], ones16[:1, :], bfc_t[:1, :ncols],
                    start=True, stop=True,
                )
                bfcs = p2r.tile([128, 512], f32, tag="bfcs")
                nc.vector.tensor_copy(bfcs[:, :ncols], pbc[:, :ncols])
                for mt in range((ntok + 127) // 128):
                    mrows = min(128, ntok - 128 * mt)
                    pl = ps2.tile([128, 512], f32, tag="pl", bufs=5)
                    for kt in range(4):
                        nc.tensor.matmul(
                            pl[:mrows, :ncols],
                            att[kt][:, 128 * mt : 128 * mt + mrows],
                            wf[:, kt, :ncols],
                            start=(kt == 0),
                            stop=(kt == 3),
                        )
                    # bias folded into the PSUM->SBUF copy (per-column bcast add)
                    ot = p2r.tile([128, 512], f32, tag="ot")
                    nc.vector.scalar_tensor_tensor(
                        ot[:mrows, :ncols], pl[:mrows, :ncols], 1.0,
                        bfcs[:mrows, :ncols], op0=OP.mult, op1=OP.add,
                    )
                    nc.scalar.dma_start(
                        out_d[128 * mt : 128 * mt + mrows, n0 : n0 + ncols],
                        ot[:mrows, :ncols],
                    )

            ps2b_cm.__exit__(None, None, None)

        dram_cm.__exit__(None, None, None)
        per_cm.__exit__(None, None, None)

    nc.compile()
    return nc


def _shard_inputs(inputs, memory, enc_h, enc_c, E, Wm, W_lstm, U_lstm, b_lstm, Wa, Wfc, bfc):
    inputs = np.ascontiguousarray(inputs)
    # h2 = 2h convention: pre-halve everything h multiplies, double the carries
    Wa_mod = np.concatenate([0.5 * Wa[:U], Wa[U:]], axis=0)
    shared = {
        "E": np.ascontiguousarray(E, np.float32),
        "Wm": np.ascontiguousarray(0.5 * Wm, np.float32),
        "W1": np.ascontiguousarray(W_lstm[:D], np.float32),
        "W2": np.ascontiguousarray(W_lstm[D:], np.float32),
        "Ul": np.ascontiguousarray(0.5 * U_lstm, np.float32),
        "bl": np.ascontiguousarray(b_lstm.reshape(1, G), np.float32),
        "Wa": np.ascontiguousarray(Wa_mod, np.float32),
        "Wfc": np.ascontiguousarray(Wfc, np.float32),
        "bfc": np.ascontiguousarray(bfc.reshape(1, VO), np.float32),
    }
    in_maps = []
    for rk in range(NCORES):
        sl = slice(BL * rk, BL * (rk + 1))
        m = dict(shared)
        m["tok_ids"] = np.ascontiguousarray(
            inputs[sl].T.reshape(NTOK, 1), np.int32
        )
        m["mem"] = np.ascontiguousarray(memory[sl], np.float32)
        m["enc_ht"] = np.ascontiguousarray(2.0 * enc_h[sl].T, np.float32)
        m["enc_c"] = np.ascontiguousarray(2.0 * enc_c[sl], np.float32)
        in_maps.append(m)
    return in_maps


def kernel(**inputs):
    from concourse.bass_utils import run_bass_kernel_spmd

    if "nc" not in _cache:
        _cache["nc"] = _build(T)
    nc = _cache["nc"]

    in_maps = _shard_inputs(**inputs)
    res = run_bass_kernel_spmd(nc, in_maps, core_ids=list(range(NCORES)))
    outs = []
    for rk in range(NCORES):
        o = res.results[rk]["out"]  # [512, 32001], rows (t, b)
        outs.append(o.reshape(T, BL, VO).transpose(1, 0, 2))
    full = np.concatenate(outs, axis=0)  # [32, 128, 32001]
    return full.astype(np.float32)
